# revision 2
# baseline (speedup 1.0000x reference)
"""Bass/Tile MHA kernel for TRN2 — per-core program, v3.

v2 + scheduling/engine-balance work:
  - scores k-tiles paired into [128,1024] PSUM tiles; one exp per pair
    (halves Act per-instruction overhead, the local bottleneck)
  - causal mask-mult and softmax normalization (normalize_recip) on the
    otherwise-idle GPSIMD/Pool engine; po PSUM freed by one DVE copy
  - B(0) prologue and D(3) tail draw PSUM from the double-buffered pS
    pool instead of the single-buffered filler pool
  - DMA priority order: wq+x(sc0) interleaved, then wk, wv, rest
See kernel_v2.py docstring for the algorithm itself.
"""

from contextlib import ExitStack

import numpy as np

import concourse.bass as bass
import concourse.bacc as bacc
import concourse.mybir as mybir
import concourse.tile as tile

F32 = mybir.dt.float32
BF16 = mybir.dt.bfloat16
ADD = mybir.AluOpType.add
MULT = mybir.AluOpType.mult
EXP = mybir.ActivationFunctionType.Exp

S = 2048
D = 1024
JC = 512
DK = 64
NH = 8
NSC = 4
NST = 16
ND = 8
NJ = 4
VW = 65


def build_mha():
    nc = bacc.Bacc("TRN2", target_bir_lowering=False, debug=False)

    xT = nc.dram_tensor("xT", [D, S], BF16, kind="ExternalInput").ap()
    wq = nc.dram_tensor("wq", [D, JC], BF16, kind="ExternalInput").ap()
    wk = nc.dram_tensor("wk", [D, JC], BF16, kind="ExternalInput").ap()
    wv = nc.dram_tensor("wv", [D, JC], BF16, kind="ExternalInput").ap()
    wo = nc.dram_tensor("wo", [JC, D], BF16, kind="ExternalInput").ap()
    bq = nc.dram_tensor("bq", [128, NJ], F32, kind="ExternalInput").ap()
    bk = nc.dram_tensor("bk", [128, NJ], F32, kind="ExternalInput").ap()
    bvb = nc.dram_tensor("bvb", [128, JC], F32, kind="ExternalInput").ap()
    mask = nc.dram_tensor("mask", [128, 128], BF16, kind="ExternalInput").ap()
    ident = nc.dram_tensor("ident", [128, 128], BF16, kind="ExternalInput").ap()
    out = nc.dram_tensor("out", [S, D], BF16, kind="ExternalOutput").ap()

    with tile.TileContext(nc) as tc, ExitStack() as ctx:
        const = ctx.enter_context(tc.tile_pool(name="const", bufs=1))
        bq_t = const.tile([128, NJ], F32)
        bk_t = const.tile([128, NJ], F32)
        bvb_t = const.tile([128, JC], F32)
        mask_t = const.tile([128, 128], BF16)
        ident_t = const.tile([128, 128], BF16)

        wp = ctx.enter_context(tc.tile_pool(name="wts", bufs=1))
        wq_t = wp.tile([128, ND * JC], BF16)
        wk_t = wp.tile([128, ND * JC], BF16)
        wv_t = wp.tile([128, ND * JC], BF16)
        wo_t = wp.tile([128, NJ * D], BF16)

        big = ctx.enter_context(tc.tile_pool(name="big", bufs=1))
        xT_t = big.tile([128, ND * S], BF16, tag="xT")
        qT_t = big.tile([128, NJ * S], BF16, tag="qT")
        kT_t = big.tile([128, NJ * S], BF16, tag="kT")
        v_t = big.tile([128, NST * NH * VW], BF16, tag="v")
        oT_t = big.tile([128, NJ * S], BF16, tag="oT")

        # psum: pS 3x2 + pV 2 = 8 banks (everything rotates through pS)
        pS = ctx.enter_context(tc.tile_pool(name="pS", bufs=3, space="PSUM"))
        pV = ctx.enter_context(tc.tile_pool(name="pV", bufs=2, space="PSUM"))

        ep = ctx.enter_context(tc.tile_pool(name="exp", bufs=8))
        pf = ctx.enter_context(tc.tile_pool(name="pof", bufs=4))
        ap_ = ctx.enter_context(tc.tile_pool(name="anorm", bufs=6))
        ostp = ctx.enter_context(tc.tile_pool(name="ost", bufs=3))

        # ---- DMAs: weights via SWDGE (gpsimd, idle Pool engine) in halves,
        # x via HWDGE (sync) per-chunk — two parallel DGE paths, ordered so
        # the first Q accumulation starts ~4us in.
        xT_r = xT_t[:].rearrange("p (c s) -> p c s", c=ND)
        xT_d = xT.rearrange("(c p) s -> p c s", p=128)
        wq_r = wq_t[:].rearrange("p (c j) -> p c j", c=ND)
        wk_r = wk_t[:].rearrange("p (c j) -> p c j", c=ND)
        wv_r = wv_t[:].rearrange("p (c j) -> p c j", c=ND)

        wq_d = wq.rearrange("(c p) j -> p c j", p=128)
        wk_d = wk.rearrange("(c p) j -> p c j", p=128)
        wv_d = wv.rearrange("(c p) j -> p c j", p=128)
        nc.gpsimd.dma_start(wq_r[:, 0:1], wq_d[:, 0:1])
        nc.sync.dma_start(xT_r[:, 0, 0:512], xT_d[:, 0, 0:512])
        nc.gpsimd.dma_start(wq_r[:, 1:2], wq_d[:, 1:2])
        nc.sync.dma_start(xT_r[:, 1, 0:512], xT_d[:, 1, 0:512])
        nc.gpsimd.dma_start(wq_r[:, 2:4], wq_d[:, 2:4])
        for dc in range(2, 4):
            nc.sync.dma_start(xT_r[:, dc, 0:512], xT_d[:, dc, 0:512])
        nc.gpsimd.dma_start(wq_r[:, 4:8], wq_d[:, 4:8])
        for dc in range(4, ND):
            nc.sync.dma_start(xT_r[:, dc, 0:512], xT_d[:, dc, 0:512])
        nc.sync.dma_start(bq_t[:], bq[:, :])
        nc.gpsimd.dma_start(wk_r[:, 0:4], wk_d[:, 0:4])
        nc.gpsimd.dma_start(wk_r[:, 4:8], wk_d[:, 4:8])
        nc.sync.dma_start(bk_t[:], bk[:, :])
        nc.gpsimd.dma_start(wv_r[:, 0:4], wv_d[:, 0:4])
        nc.gpsimd.dma_start(wv_r[:, 4:8], wv_d[:, 4:8])
        nc.sync.dma_start(bvb_t[:], bvb[:, :])
        nc.sync.dma_start(mask_t[:], mask[:, :])
        nc.sync.dma_start(ident_t[:], ident[:, :])
        for sc in range(1, NSC):
            nc.sync.dma_start(xT_r[:, :, sc * 512:(sc + 1) * 512],
                              xT_d[:, :, sc * 512:(sc + 1) * 512])
        nc.gpsimd.dma_start(wo_t[:].rearrange("p (c j) -> p c j", c=NJ),
                            wo.rearrange("(c p) j -> p c j", p=128))

        v_r = v_t[:].rearrange("p (st h w) -> p st h w", st=NST, h=NH)
        bvb_r = bvb_t[:].rearrange("p (h w) -> p h w", h=NH)

        # PE pstate warmup: ~3us of dummy matmuls while the first DMAs land
        with tc.tile_pool(name="warm", bufs=1) as wmp:
            wm = wmp.tile([128, 512], BF16, tag="wm", name="wm")
            nc.vector.memset(wm[:], 0.0)
            pw = pS.tile([128, 1024], F32, tag="sc", name="pw")
            for i in range(14):
                nc.tensor.matmul(pw[:, 0:512], wm[:, 0:128], wm[:],
                                 start=(i == 0), stop=(i == 13))

        # big-tile half allocator (pS): yields [128,512] f32 psum views
        _half = [None, 0]

        def ps_half():
            if _half[1] % 2 == 0:
                _half[0] = pS.tile([128, 1024], F32, tag="sc", name="psh")
            _half[1] += 1
            o = 512 * ((_half[1] - 1) % 2)
            return _half[0][:, o:o + 512]

        def ps_quads():
            t1 = pS.tile([128, 1024], F32, tag="sc", name="psq1")
            t2 = pS.tile([128, 1024], F32, tag="sc", name="psq2")
            return [t1[:, 0:512], t1[:, 512:1024],
                    t2[:, 0:512], t2[:, 512:1024]]

        # B(0) prologue: dc-outer with 4 concurrent psum groups, so compute
        # starts as soon as the first weight/x chunk lands.
        def b0_qk(w_t, b_t, dstT):
            hv = ps_quads()
            for dc in range(ND):
                for jt in range(NJ):
                    nc.tensor.matmul(
                        hv[jt],
                        w_t[:, dc * JC + jt * 128: dc * JC + (jt + 1) * 128],
                        xT_t[:, dc * S: dc * S + 512],
                        start=(dc == 0), stop=(dc == ND - 1))
            for jt in range(NJ):
                nc.vector.tensor_scalar(
                    dstT[:, jt * S: jt * S + 512],
                    hv[jt], b_t[:, jt:jt + 1], None, op0=ADD)

        def b0_v():
            hv = ps_quads()
            for dc in range(ND):
                for st in range(4):
                    nc.tensor.matmul(
                        hv[st],
                        xT_t[:, dc * S + st * 128: dc * S + (st + 1) * 128],
                        wv_t[:, dc * JC:(dc + 1) * JC],
                        start=(dc == 0), stop=(dc == ND - 1))
            for st in range(4):
                nc.vector.tensor_tensor(
                    v_r[:, st, :, 0:DK],
                    hv[st].rearrange("p (h w) -> p h w", h=NH), bvb_r[:],
                    op=ADD)
                nc.gpsimd.memset(v_r[:, st, :, DK:VW], 1.0)

        # ---- phase B units
        def unit_qk(w_t, b_t, dstT, sc, jt, alloc, half):
            dst = None

            def emit():
                pq = alloc()
                dcs = range(4) if half == 0 else range(4, ND)
                for dc in dcs:
                    nc.tensor.matmul(
                        pq,
                        w_t[:, dc * JC + jt * 128: dc * JC + (jt + 1) * 128],
                        xT_t[:, dc * S + sc * 512: dc * S + (sc + 1) * 512],
                        start=(dc == dcs[0]), stop=(dc == dcs[-1]))
                d = dstT[:, jt * S + sc * 512: jt * S + (sc + 1) * 512]
                if half == 0:
                    nc.vector.tensor_scalar(
                        d, pq, b_t[:, jt:jt + 1], None, op0=ADD)
                else:
                    nc.vector.tensor_tensor(d, d, pq, op=ADD)
            return emit

        def unit_v(sc, st4, alloc):
            def emit():
                st = sc * 4 + st4
                pv = alloc()
                for dc in range(ND):
                    nc.tensor.matmul(
                        pv,
                        xT_t[:, dc * S + st * 128: dc * S + (st + 1) * 128],
                        wv_t[:, dc * JC:(dc + 1) * JC],
                        start=(dc == 0), stop=(dc == ND - 1))
                nc.vector.tensor_tensor(
                    v_r[:, st, :, 0:DK],
                    pv.rearrange("p (h w) -> p h w", h=NH), bvb_r[:],
                    op=ADD)
                nc.gpsimd.memset(v_r[:, st, :, DK:VW], 1.0)
            return emit

        def b_units(sc, alloc):
            us = []
            for jt in range(NJ):
                for half in range(2):
                    us.append(unit_qk(wq_t, bq_t, qT_t, sc, jt, alloc, half))
            for jt in range(NJ):
                for half in range(2):
                    us.append(unit_qk(wk_t, bk_t, kT_t, sc, jt, alloc, half))
            for st4 in range(4):
                us.append(unit_v(sc, st4, alloc))
            return us

        # ---- phase D half-units: one (st, od) psum group each
        def d_units(sc, alloc):
            us = []
            for st in range(sc * 4, sc * 4 + 4):
                ot = [None]

                def emit(st=st, ot=ot):
                    ot[0] = ostp.tile([128, D], BF16, tag="ost", name="ot")
                    pd = alloc()
                    for vc in range(NJ):
                        nc.tensor.matmul(
                            pd,
                            oT_t[:, vc * S + st * 128: vc * S + (st + 1) * 128],
                            wo_t[:, vc * D: vc * D + 512],
                            start=(vc == 0), stop=(vc == NJ - 1))
                    nc.vector.tensor_copy(ot[0][:, 0:512], pd)
                    nc.sync.dma_start(out[st * 128:(st + 1) * 128, 0:512],
                                      ot[0][:, 0:512])

                def emit2(st=st, ot=ot):
                    pd = alloc()
                    for vc in range(NJ):
                        nc.tensor.matmul(
                            pd,
                            oT_t[:, vc * S + st * 128: vc * S + (st + 1) * 128],
                            wo_t[:, vc * D + 512: vc * D + 1024],
                            start=(vc == 0), stop=(vc == NJ - 1))
                    nc.vector.tensor_copy(ot[0][:, 512:1024], pd)
                    nc.sync.dma_start(out[st * 128:(st + 1) * 128, 512:1024],
                                      ot[0][:, 512:1024])

                us.append(emit)
                us.append(emit2)
            return us

        # ---- phase C
        pending_tr = [None]   # deferred transpose emitter (1-pair pipeline)

        def flush_tr():
            if pending_tr[0] is not None:
                pending_tr[0]()
                pending_tr[0] = None

        def phase_c(qt, units):
            n_pair = NH * (2 * qt + 2)
            frac = len(units) / n_pair if units else 0.0
            ui = iter(units)
            acc = 0.0
            a_pair = [None, None]
            po_h = {}
            av_q = []

            def make_tr(a0, a1, hp):
                def emit_tr():
                    pt = pV.tile([128, 512], BF16, tag="po",
                                 name="pt", padded_shape=[128, 1024])
                    for g, ag in enumerate((a0, a1)):
                        for qsub in range(4):
                            nc.tensor.transpose(
                                pt[g * 64:(g + 1) * 64,
                                   qsub * 128:(qsub + 1) * 128],
                                ag[:, qsub * DK:(qsub + 1) * DK],
                                ident_t[:])
                    if qt == 3 and hp == NJ - 1:
                        for q4 in range(4):   # quartered: D(3) starts sooner
                            nc.vector.tensor_copy(
                                oT_t[:, hp * S + qt * 512 + q4 * 128:
                                     hp * S + qt * 512 + (q4 + 1) * 128],
                                pt[:, q4 * 128:(q4 + 1) * 128])
                    else:
                        nc.vector.tensor_copy(
                            oT_t[:, hp * S + qt * 512:
                                 hp * S + (qt + 1) * 512],
                            pt[:, 0:512])
                return emit_tr

            def emit_norm(h):
                hp, h2 = h // 2, h % 2
                po = po_h.pop(h)
                a = ap_.tile([128, 4 * DK], BF16, tag="a", name="a")
                if qt == 3 and h == NH - 1:
                    # tail: normalize straight from PSUM on DVE
                    po_r = po[:].rearrange("p (q w) -> p q w", q=4)
                    r = pf.tile([128, 4], F32, tag="r", name="r")
                    nc.vector.reciprocal(r[:], po_r[:, :, DK])
                    for qsub in range(4):
                        nc.vector.tensor_scalar(
                            a[:, qsub * DK:(qsub + 1) * DK],
                            po_r[:, qsub, 0:DK], r[:, qsub:qsub + 1], None,
                            op0=MULT)
                else:
                    pof = pf.tile([128, 4 * VW], F32, tag="pof", name="pof")
                    nc.vector.tensor_copy(pof[:], po[:])
                    pof_r = pof[:].rearrange("p (q w) -> p q w", q=4)
                    for qsub in range(4):
                        nc.gpsimd.normalize_recip(
                            a[:, qsub * DK:(qsub + 1) * DK],
                            pof_r[:, qsub, 0:DK], pof_r[:, qsub, DK:VW])
                a_pair[h2] = a
                if h2 == 1:
                    tr = make_tr(a_pair[0], a_pair[1], hp)
                    flush_tr()          # previous pair, one pair deferred
                    pending_tr[0] = tr

            for h in range(NH):
                hp, h2 = h // 2, h % 2
                lo = 64 * h2
                qcol = hp * S + qt * 512
                po = pV.tile([128, 4 * VW], F32, tag="po", name="po",
                             padded_shape=[128, 512])
                po_h[h] = po
                for kp in range(2 * qt + 2):
                    ps = pS.tile([128, 1024], F32, tag="sc", name="ps")
                    e = ep.tile([128, 1024], BF16, tag="e", name="e")
                    offs = []
                    for k2 in range(2):
                        kt = 2 * kp + k2
                        dd = kt - 4 * qt
                        off = max(dd, 0) * 128
                        offs.append(off)
                        nc.tensor.matmul(
                            ps[:, k2 * 512 + off: (k2 + 1) * 512],
                            kT_t[lo:lo + 64,
                                 hp * S + kt * 128: hp * S + (kt + 1) * 128],
                            qT_t[lo:lo + 64, qcol + off: qcol + 512],
                            start=True, stop=True)
                    if offs[1] == 128:
                        # diag pair 1: single exp over both halves; cols
                        # [512,640) are stale psum, finite, and never read
                        nc.scalar.activation(e[:], ps[:], EXP, scale=0.125)
                    elif offs[0] == 0:
                        nc.scalar.activation(e[:], ps[:], EXP, scale=0.125)
                    else:
                        for k2 in range(2):
                            o = k2 * 512 + offs[k2]
                            nc.scalar.activation(
                                e[:, o:(k2 + 1) * 512], ps[:, o:(k2 + 1) * 512],
                                EXP, scale=0.125)
                    for k2 in range(2):
                        kt = 2 * kp + k2
                        dd = kt - 4 * qt
                        if dd >= 0:
                            o = k2 * 512 + dd * 128
                            nc.gpsimd.tensor_tensor(
                                e[:, o:o + 128], e[:, o:o + 128],
                                mask_t[:], op=MULT)

                    def make_av(e=e, kp=kp, h=h, po=po):
                        def emit_av():
                            for k2 in range(2):
                                kt = 2 * kp + k2
                                dd = kt - 4 * qt
                                for qsub in range(max(dd, 0), 4):
                                    nc.tensor.matmul(
                                        po[:, qsub * VW: qsub * VW + VW],
                                        e[:, k2 * 512 + qsub * 128:
                                           k2 * 512 + (qsub + 1) * 128],
                                        v_t[:, (kt * NH + h) * VW:
                                            (kt * NH + h + 1) * VW],
                                        start=(kt == 0 and qsub == 0),
                                        stop=(kt == 4 * qt + 3 and qsub == 3),
                                        skip_group_check=True)
                        return emit_av
                    if len(av_q) >= (3 if qt >= 1 else 2):
                        av_q.pop(0)()
                    if kp == (2 if qt >= 1 else 1) and h > 0:
                        emit_norm(h - 1)
                    av_q.append(make_av())
                    acc += frac
                    while acc >= 1.0:
                        acc -= 1.0
                        u = next(ui, None)
                        if u is not None:
                            u()
            for av in av_q:
                av()
            av_q.clear()
            emit_norm(NH - 1)
            for u in ui:
                u()

        # ---- main schedule
        def ps_full():
            return pS.tile([128, 1024], F32, tag="sc", name="pp")[:, 0:512]

        b0_qk(wq_t, bq_t, qT_t)
        b0_qk(wk_t, bk_t, kT_t)
        b0_v()
        for qt in range(NSC):
            if qt < 3:
                units = b_units(qt + 1, ps_full)
            else:
                units = (d_units(0, ps_full) + d_units(1, ps_full)
                         + d_units(2, ps_full))
            phase_c(qt, units)
        flush_tr()
        for u in d_units(3, ps_half):
            u()

    nc.compile()
    return nc


# ----------------------------------------------------------------- host side

_NC_CACHE = None


def _get_nc():
    global _NC_CACHE
    if _NC_CACHE is None:
        _NC_CACHE = build_mha()
    return _NC_CACHE


def shard_inputs(x, Wq, bq, Wk, bk, Wv, bv, Wo, bo):
    import ml_dtypes
    bf16 = ml_dtypes.bfloat16
    mask = np.triu(np.ones((128, 128), np.float32)).astype(bf16)
    ident = np.eye(128, dtype=np.float32).astype(bf16)
    x = np.asarray(x, dtype=np.float32)
    xTs = [np.ascontiguousarray(x[b].T).astype(bf16) for b in range(4)]
    Wq, Wk, Wv, Wo = (np.asarray(w, np.float32) for w in (Wq, Wk, Wv, Wo))
    bq, bk, bv = (np.asarray(b_, np.float32) for b_ in (bq, bk, bv))
    maps = []
    for c in range(8):
        b, g = c // 2, c % 2
        sl = slice(g * JC, (g + 1) * JC)
        maps.append({
            "xT": xTs[b],
            "wq": np.ascontiguousarray(Wq[:, sl]).astype(bf16),
            "wk": np.ascontiguousarray(Wk[:, sl]).astype(bf16),
            "wv": np.ascontiguousarray(Wv[:, sl]).astype(bf16),
            "wo": np.ascontiguousarray(Wo[sl, :]).astype(bf16),
            "bq": np.ascontiguousarray(bq[sl].reshape(NJ, 128).T),
            "bk": np.ascontiguousarray(bk[sl].reshape(NJ, 128).T),
            "bvb": np.broadcast_to(bv[sl], (128, JC)).copy(),
            "mask": mask,
            "ident": ident,
        })
    return maps


def kernel(x, Wq, bq, Wk, bk, Wv, bv, Wo, bo):
    """Full-input entry point: shard across 8 NeuronCores, run, gather."""
    from concourse.bass_utils import run_bass_kernel_spmd

    nc = _get_nc()
    in_maps = shard_inputs(x, Wq, bq, Wk, bk, Wv, bv, Wo, bo)
    res = run_bass_kernel_spmd(nc, in_maps, list(range(8)))
    bo = np.asarray(bo, dtype=np.float32)
    out = np.empty((4, S, D), dtype=np.float32)
    for b in range(4):
        out[b] = (res.results[2 * b]["out"].astype(np.float32)
                  + res.results[2 * b + 1]["out"].astype(np.float32) + bo)
    return out


# revision 6
# speedup vs baseline: 1.0418x; 1.0418x over previous
"""Bass/Tile MHA kernel for TRN2 — per-core program, v3.

v2 + scheduling/engine-balance work:
  - scores k-tiles paired into [128,1024] PSUM tiles; one exp per pair
    (halves Act per-instruction overhead, the local bottleneck)
  - causal mask-mult and softmax normalization (normalize_recip) on the
    otherwise-idle GPSIMD/Pool engine; po PSUM freed by one DVE copy
  - B(0) prologue and D(3) tail draw PSUM from the double-buffered pS
    pool instead of the single-buffered filler pool
  - DMA priority order: wq+x(sc0) interleaved, then wk, wv, rest
See kernel_v2.py docstring for the algorithm itself.
"""

from contextlib import ExitStack

import numpy as np

import concourse.bass as bass
import concourse.bacc as bacc
import concourse.mybir as mybir
import concourse.tile as tile

F32 = mybir.dt.float32
BF16 = mybir.dt.bfloat16
ADD = mybir.AluOpType.add
MULT = mybir.AluOpType.mult
EXP = mybir.ActivationFunctionType.Exp

S = 2048
D = 1024
JC = 512
DK = 64
NH = 8
NSC = 4
NST = 16
ND = 8
NJ = 4
VW = 65


def build_mha():
    nc = bacc.Bacc("TRN2", target_bir_lowering=False, debug=False)

    xT = nc.dram_tensor("xT", [D, S], BF16, kind="ExternalInput").ap()
    wq = nc.dram_tensor("wq", [D, JC], BF16, kind="ExternalInput").ap()
    wk = nc.dram_tensor("wk", [D, JC], BF16, kind="ExternalInput").ap()
    wv = nc.dram_tensor("wv", [D, JC], BF16, kind="ExternalInput").ap()
    wo = nc.dram_tensor("wo", [JC, D], BF16, kind="ExternalInput").ap()
    bq = nc.dram_tensor("bq", [128, NJ], F32, kind="ExternalInput").ap()
    bk = nc.dram_tensor("bk", [128, NJ], F32, kind="ExternalInput").ap()
    bvb = nc.dram_tensor("bvb", [128, JC], F32, kind="ExternalInput").ap()
    mask = nc.dram_tensor("mask", [128, 128], BF16, kind="ExternalInput").ap()
    ident = nc.dram_tensor("ident", [128, 128], BF16, kind="ExternalInput").ap()
    out = nc.dram_tensor("out", [S, D], BF16, kind="ExternalOutput").ap()

    with tile.TileContext(nc) as tc, ExitStack() as ctx:
        const = ctx.enter_context(tc.tile_pool(name="const", bufs=1))
        bq_t = const.tile([128, NJ], F32)
        bk_t = const.tile([128, NJ], F32)
        bvb_t = const.tile([128, JC], F32)
        mask_t = const.tile([128, 128], BF16)
        ident_t = const.tile([128, 128], BF16)

        wp = ctx.enter_context(tc.tile_pool(name="wts", bufs=1))
        wq_t = wp.tile([128, ND * JC], BF16)
        wk_t = wp.tile([128, ND * JC], BF16)
        wv_t = wp.tile([128, ND * JC], BF16)
        wo_t = wp.tile([128, NJ * D], BF16)

        big = ctx.enter_context(tc.tile_pool(name="big", bufs=1))
        xT_t = big.tile([128, ND * S], BF16, tag="xT")
        qT_t = big.tile([128, NJ * S], BF16, tag="qT")
        kT_t = big.tile([128, NJ * S], BF16, tag="kT")
        v_t = big.tile([128, NST * NH * VW], BF16, tag="v")
        oT_t = big.tile([128, NJ * S], BF16, tag="oT")

        # psum: pS 3x2 + pV 2 = 8 banks (everything rotates through pS)
        pS = ctx.enter_context(tc.tile_pool(name="pS", bufs=3, space="PSUM"))
        pV = ctx.enter_context(tc.tile_pool(name="pV", bufs=2, space="PSUM"))

        ep = ctx.enter_context(tc.tile_pool(name="exp", bufs=8))
        pf = ctx.enter_context(tc.tile_pool(name="pof", bufs=4))
        ap_ = ctx.enter_context(tc.tile_pool(name="anorm", bufs=6))
        ostp = ctx.enter_context(tc.tile_pool(name="ost", bufs=6))

        # ---- DMAs: weights via SWDGE (gpsimd, idle Pool engine) in halves,
        # x via HWDGE (sync) per-chunk — two parallel DGE paths, ordered so
        # the first Q accumulation starts ~4us in.
        xT_r = xT_t[:].rearrange("p (c s) -> p c s", c=ND)
        xT_d = xT.rearrange("(c p) s -> p c s", p=128)
        wq_r = wq_t[:].rearrange("p (c j) -> p c j", c=ND)
        wk_r = wk_t[:].rearrange("p (c j) -> p c j", c=ND)
        wv_r = wv_t[:].rearrange("p (c j) -> p c j", c=ND)

        wq_d = wq.rearrange("(c p) j -> p c j", p=128)
        wk_d = wk.rearrange("(c p) j -> p c j", p=128)
        wv_d = wv.rearrange("(c p) j -> p c j", p=128)
        nc.gpsimd.dma_start(wq_r[:, 0:1], wq_d[:, 0:1])
        nc.sync.dma_start(xT_r[:, 0, 0:512], xT_d[:, 0, 0:512])
        nc.gpsimd.dma_start(wq_r[:, 1:2], wq_d[:, 1:2])
        nc.sync.dma_start(xT_r[:, 1, 0:512], xT_d[:, 1, 0:512])
        nc.gpsimd.dma_start(wq_r[:, 2:4], wq_d[:, 2:4])
        for dc in range(2, 4):
            nc.sync.dma_start(xT_r[:, dc, 0:512], xT_d[:, dc, 0:512])
        nc.gpsimd.dma_start(wq_r[:, 4:8], wq_d[:, 4:8])
        for dc in range(4, ND):
            nc.sync.dma_start(xT_r[:, dc, 0:512], xT_d[:, dc, 0:512])
        nc.sync.dma_start(bq_t[:], bq[:, :])
        nc.gpsimd.dma_start(wk_r[:, 0:4], wk_d[:, 0:4])
        nc.gpsimd.dma_start(wk_r[:, 4:8], wk_d[:, 4:8])
        nc.sync.dma_start(bk_t[:], bk[:, :])
        nc.gpsimd.dma_start(wv_r[:, 0:4], wv_d[:, 0:4])
        nc.gpsimd.dma_start(wv_r[:, 4:8], wv_d[:, 4:8])
        nc.sync.dma_start(bvb_t[:], bvb[:, :])
        nc.sync.dma_start(mask_t[:], mask[:, :])
        nc.sync.dma_start(ident_t[:], ident[:, :])
        for sc in range(1, NSC):
            nc.sync.dma_start(xT_r[:, :, sc * 512:(sc + 1) * 512],
                              xT_d[:, :, sc * 512:(sc + 1) * 512])
        nc.gpsimd.dma_start(wo_t[:].rearrange("p (c j) -> p c j", c=NJ),
                            wo.rearrange("(c p) j -> p c j", p=128))

        v_r = v_t[:].rearrange("p (st h w) -> p st h w", st=NST, h=NH)
        bvb_r = bvb_t[:].rearrange("p (h w) -> p h w", h=NH)

        # PE pstate warmup: ~3us of dummy matmuls while the first DMAs land
        with tc.tile_pool(name="warm", bufs=1) as wmp:
            wm = wmp.tile([128, 512], BF16, tag="wm", name="wm")
            nc.vector.memset(wm[:], 0.0)
            pw = pS.tile([128, 1024], F32, tag="sc", name="pw")
            for i in range(14):
                nc.tensor.matmul(pw[:, 0:512], wm[:, 0:128], wm[:],
                                 start=(i == 0), stop=(i == 13))

        # big-tile half allocator (pS): yields [128,512] f32 psum views
        _half = [None, 0]

        def ps_half():
            if _half[1] % 2 == 0:
                _half[0] = pS.tile([128, 1024], F32, tag="sc", name="psh")
            _half[1] += 1
            o = 512 * ((_half[1] - 1) % 2)
            return _half[0][:, o:o + 512]

        def ps_quads():
            t1 = pS.tile([128, 1024], F32, tag="sc", name="psq1")
            t2 = pS.tile([128, 1024], F32, tag="sc", name="psq2")
            return [t1[:, 0:512], t1[:, 512:1024],
                    t2[:, 0:512], t2[:, 512:1024]]

        # B(0) prologue: dc-outer with 4 concurrent psum groups, so compute
        # starts as soon as the first weight/x chunk lands.
        def b0_qk(w_t, b_t, dstT):
            hv = ps_quads()
            for dc in range(ND):
                for jt in range(NJ):
                    nc.tensor.matmul(
                        hv[jt],
                        w_t[:, dc * JC + jt * 128: dc * JC + (jt + 1) * 128],
                        xT_t[:, dc * S: dc * S + 512],
                        start=(dc == 0), stop=(dc == ND - 1))
            for jt in range(NJ):
                nc.vector.tensor_scalar(
                    dstT[:, jt * S: jt * S + 512],
                    hv[jt], b_t[:, jt:jt + 1], None, op0=ADD)

        def b0_v():
            for st in range(4):
                pv = ps_full()
                for dc in range(ND):
                    nc.tensor.matmul(
                        pv,
                        xT_t[:, dc * S + st * 128: dc * S + (st + 1) * 128],
                        wv_t[:, dc * JC:(dc + 1) * JC],
                        start=(dc == 0), stop=(dc == ND - 1))
                nc.vector.tensor_tensor(
                    v_r[:, st, :, 0:DK],
                    pv.rearrange("p (h w) -> p h w", h=NH), bvb_r[:],
                    op=ADD)
                nc.gpsimd.memset(v_r[:, st, :, DK:VW], 1.0)

        # ---- phase B units
        def unit_qk(w_t, b_t, dstT, sc, jt, alloc, half):
            dst = None

            def emit():
                pq = alloc()
                dcs = range(4) if half == 0 else range(4, ND)
                for dc in dcs:
                    nc.tensor.matmul(
                        pq,
                        w_t[:, dc * JC + jt * 128: dc * JC + (jt + 1) * 128],
                        xT_t[:, dc * S + sc * 512: dc * S + (sc + 1) * 512],
                        start=(dc == dcs[0]), stop=(dc == dcs[-1]))
                d = dstT[:, jt * S + sc * 512: jt * S + (sc + 1) * 512]
                if half == 0:
                    nc.vector.tensor_scalar(
                        d, pq, b_t[:, jt:jt + 1], None, op0=ADD)
                else:
                    nc.vector.tensor_tensor(d, d, pq, op=ADD)
            return emit

        def unit_v(sc, st4, alloc):
            def emit():
                st = sc * 4 + st4
                pv = alloc()
                for dc in range(ND):
                    nc.tensor.matmul(
                        pv,
                        xT_t[:, dc * S + st * 128: dc * S + (st + 1) * 128],
                        wv_t[:, dc * JC:(dc + 1) * JC],
                        start=(dc == 0), stop=(dc == ND - 1))
                nc.vector.tensor_tensor(
                    v_r[:, st, :, 0:DK],
                    pv.rearrange("p (h w) -> p h w", h=NH), bvb_r[:],
                    op=ADD)
                nc.gpsimd.memset(v_r[:, st, :, DK:VW], 1.0)
            return emit

        def b_units(sc, alloc):
            us = []
            for jt in range(NJ):
                for half in range(2):
                    us.append(unit_qk(wq_t, bq_t, qT_t, sc, jt, alloc, half))
            for jt in range(NJ):
                for half in range(2):
                    us.append(unit_qk(wk_t, bk_t, kT_t, sc, jt, alloc, half))
            for st4 in range(4):
                us.append(unit_v(sc, st4, alloc))
            return us

        # ---- phase D half-units: one (st, od) psum group each
        def d_units(sc, alloc):
            us = []
            for st in range(sc * 4, sc * 4 + 4):
                ot = [None]

                def emit(st=st, ot=ot):
                    ot[0] = ostp.tile([128, D], BF16, tag="ost", name="ot")
                    pd = alloc()
                    for vc in range(NJ):
                        nc.tensor.matmul(
                            pd,
                            oT_t[:, vc * S + st * 128: vc * S + (st + 1) * 128],
                            wo_t[:, vc * D: vc * D + 512],
                            start=(vc == 0), stop=(vc == NJ - 1))
                    nc.vector.tensor_copy(ot[0][:, 0:512], pd)
                    nc.sync.dma_start(out[st * 128:(st + 1) * 128, 0:512],
                                      ot[0][:, 0:512])

                def emit2(st=st, ot=ot):
                    pd = alloc()
                    for vc in range(NJ):
                        nc.tensor.matmul(
                            pd,
                            oT_t[:, vc * S + st * 128: vc * S + (st + 1) * 128],
                            wo_t[:, vc * D + 512: vc * D + 1024],
                            start=(vc == 0), stop=(vc == NJ - 1))
                    nc.vector.tensor_copy(ot[0][:, 512:1024], pd)
                    nc.sync.dma_start(out[st * 128:(st + 1) * 128, 512:1024],
                                      ot[0][:, 512:1024])

                us.append(emit)
                us.append(emit2)
            return us

        # ---- phase C
        pending_tr = [None]   # deferred transpose emitter (1-pair pipeline)

        def flush_tr():
            if pending_tr[0] is not None:
                pending_tr[0]()
                pending_tr[0] = None

        def phase_c(qt, units):
            n_pair = NH * (2 * qt + 2)
            frac = len(units) / n_pair if units else 0.0
            ui = iter(units)
            acc = 0.0
            a_pair = [None, None]
            po_h = {}
            av_q = []

            def make_tr(a0, a1, hp):
                def emit_tr():
                    pt = pV.tile([128, 512], BF16, tag="po",
                                 name="pt", padded_shape=[128, 1024])
                    for g, ag in enumerate((a0, a1)):
                        for qsub in range(4):
                            nc.tensor.transpose(
                                pt[g * 64:(g + 1) * 64,
                                   qsub * 128:(qsub + 1) * 128],
                                ag[:, qsub * DK:(qsub + 1) * DK],
                                ident_t[:])
                    if qt == 3 and hp == NJ - 1:
                        for q4 in range(4):   # quartered: D(3) starts sooner
                            nc.vector.tensor_copy(
                                oT_t[:, hp * S + qt * 512 + q4 * 128:
                                     hp * S + qt * 512 + (q4 + 1) * 128],
                                pt[:, q4 * 128:(q4 + 1) * 128])
                    else:
                        nc.vector.tensor_copy(
                            oT_t[:, hp * S + qt * 512:
                                 hp * S + (qt + 1) * 512],
                            pt[:, 0:512])
                return emit_tr

            def emit_norm(h):
                hp, h2 = h // 2, h % 2
                po = po_h.pop(h)
                a = ap_.tile([128, 4 * DK], BF16, tag="a", name="a")
                if qt == 3 and h == NH - 1:
                    # tail: normalize straight from PSUM on DVE
                    po_r = po[:].rearrange("p (q w) -> p q w", q=4)
                    r = pf.tile([128, 4], F32, tag="r", name="r")
                    nc.vector.reciprocal(r[:], po_r[:, :, DK])
                    for qsub in range(4):
                        nc.vector.tensor_scalar(
                            a[:, qsub * DK:(qsub + 1) * DK],
                            po_r[:, qsub, 0:DK], r[:, qsub:qsub + 1], None,
                            op0=MULT)
                else:
                    pof = pf.tile([128, 4 * VW], F32, tag="pof", name="pof")
                    nc.vector.tensor_copy(pof[:], po[:])
                    pof_r = pof[:].rearrange("p (q w) -> p q w", q=4)
                    for qsub in range(4):
                        nc.gpsimd.normalize_recip(
                            a[:, qsub * DK:(qsub + 1) * DK],
                            pof_r[:, qsub, 0:DK], pof_r[:, qsub, DK:VW])
                a_pair[h2] = a
                if h2 == 1:
                    tr = make_tr(a_pair[0], a_pair[1], hp)
                    flush_tr()          # previous pair, one pair deferred
                    pending_tr[0] = tr

            for h in range(NH):
                hp, h2 = h // 2, h % 2
                lo = 64 * h2
                qcol = hp * S + qt * 512
                po = pV.tile([128, 4 * VW], F32, tag="po", name="po",
                             padded_shape=[128, 512])
                po_h[h] = po
                for kp in range(2 * qt + 2):
                    ps = pS.tile([128, 1024], F32, tag="sc", name="ps")
                    e = ep.tile([128, 1024], BF16, tag="e", name="e")
                    # column base of k2's q-range in ps/e: diagonal pairs
                    # are COMPACTED so one contiguous exp covers both
                    # halves with no junk columns.
                    dd0 = 2 * kp - 4 * qt
                    if dd0 == 0:          # diag pair 1: [0:512]+[512:896]
                        base = (0, 512 - 128)
                        erng = (0, 896)
                    elif dd0 == 2:        # diag pair 2: [256:512]+[512:640]
                        base = (0, 512 - 384)
                        erng = (256, 640)
                    else:                 # full pair
                        base = (0, 512)
                        erng = (0, 1024)
                    for k2 in range(2):
                        kt = 2 * kp + k2
                        off = max(kt - 4 * qt, 0) * 128
                        nc.tensor.matmul(
                            ps[:, base[k2] + off: base[k2] + 512],
                            kT_t[lo:lo + 64,
                                 hp * S + kt * 128: hp * S + (kt + 1) * 128],
                            qT_t[lo:lo + 64, qcol + off: qcol + 512],
                            start=True, stop=True)
                    nc.scalar.activation(e[:, erng[0]:erng[1]],
                                         ps[:, erng[0]:erng[1]],
                                         EXP, scale=0.125)
                    for k2 in range(2):
                        dd = 2 * kp + k2 - 4 * qt
                        if dd >= 0:
                            o = base[k2] + dd * 128
                            nc.gpsimd.tensor_tensor(
                                e[:, o:o + 128], e[:, o:o + 128],
                                mask_t[:], op=MULT)

                    def make_av(e=e, kp=kp, h=h, po=po, base=base):
                        def emit_av():
                            for k2 in range(2):
                                kt = 2 * kp + k2
                                dd = kt - 4 * qt
                                for qsub in range(max(dd, 0), 4):
                                    nc.tensor.matmul(
                                        po[:, qsub * VW: qsub * VW + VW],
                                        e[:, base[k2] + qsub * 128:
                                           base[k2] + (qsub + 1) * 128],
                                        v_t[:, (kt * NH + h) * VW:
                                            (kt * NH + h + 1) * VW],
                                        start=(kt == 0 and qsub == 0),
                                        stop=(kt == 4 * qt + 3 and qsub == 3),
                                        skip_group_check=True)
                        return emit_av
                    if len(av_q) >= (3 if qt >= 1 else 2):
                        av_q.pop(0)()
                    if kp == (2 if qt >= 1 else 1) and h > 0:
                        emit_norm(h - 1)
                    av_q.append(make_av())
                    acc += frac
                    while acc >= 1.0:
                        acc -= 1.0
                        u = next(ui, None)
                        if u is not None:
                            u()
            for av in av_q:
                av()
            av_q.clear()
            emit_norm(NH - 1)
            for u in ui:
                u()

        # ---- main schedule
        def ps_full():
            return pS.tile([128, 1024], F32, tag="sc", name="pp")[:, 0:512]

        b0_qk(wq_t, bq_t, qT_t)
        b0_qk(wk_t, bk_t, kT_t)
        b0_v()
        for qt in range(NSC):
            if qt < 3:
                units = b_units(qt + 1, ps_full)
            else:
                units = (d_units(0, ps_full) + d_units(1, ps_full)
                         + d_units(2, ps_full))
            phase_c(qt, units)
        flush_tr()
        for u in d_units(3, ps_full):
            u()

    nc.compile()
    return nc


# ----------------------------------------------------------------- host side

_NC_CACHE = None


def _get_nc():
    global _NC_CACHE
    if _NC_CACHE is None:
        _NC_CACHE = build_mha()
    return _NC_CACHE


def shard_inputs(x, Wq, bq, Wk, bk, Wv, bv, Wo, bo):
    import ml_dtypes
    bf16 = ml_dtypes.bfloat16
    mask = np.triu(np.ones((128, 128), np.float32)).astype(bf16)
    ident = np.eye(128, dtype=np.float32).astype(bf16)
    x = np.asarray(x, dtype=np.float32)
    xTs = [np.ascontiguousarray(x[b].T).astype(bf16) for b in range(4)]
    Wq, Wk, Wv, Wo = (np.asarray(w, np.float32) for w in (Wq, Wk, Wv, Wo))
    bq, bk, bv = (np.asarray(b_, np.float32) for b_ in (bq, bk, bv))
    maps = []
    for c in range(8):
        b, g = c // 2, c % 2
        sl = slice(g * JC, (g + 1) * JC)
        maps.append({
            "xT": xTs[b],
            "wq": np.ascontiguousarray(Wq[:, sl]).astype(bf16),
            "wk": np.ascontiguousarray(Wk[:, sl]).astype(bf16),
            "wv": np.ascontiguousarray(Wv[:, sl]).astype(bf16),
            "wo": np.ascontiguousarray(Wo[sl, :]).astype(bf16),
            "bq": np.ascontiguousarray(bq[sl].reshape(NJ, 128).T),
            "bk": np.ascontiguousarray(bk[sl].reshape(NJ, 128).T),
            "bvb": np.broadcast_to(bv[sl], (128, JC)).copy(),
            "mask": mask,
            "ident": ident,
        })
    return maps


def kernel(x, Wq, bq, Wk, bk, Wv, bv, Wo, bo):
    """Full-input entry point: shard across 8 NeuronCores, run, gather."""
    from concourse.bass_utils import run_bass_kernel_spmd

    nc = _get_nc()
    in_maps = shard_inputs(x, Wq, bq, Wk, bk, Wv, bv, Wo, bo)
    res = run_bass_kernel_spmd(nc, in_maps, list(range(8)))
    bo = np.asarray(bo, dtype=np.float32)
    out = np.empty((4, S, D), dtype=np.float32)
    for b in range(4):
        out[b] = (res.results[2 * b]["out"].astype(np.float32)
                  + res.results[2 * b + 1]["out"].astype(np.float32) + bo)
    return out


# revision 7
# speedup vs baseline: 1.0444x; 1.0025x over previous
"""Bass/Tile MHA kernel for TRN2 — per-core program, v3.

v2 + scheduling/engine-balance work:
  - scores k-tiles paired into [128,1024] PSUM tiles; one exp per pair
    (halves Act per-instruction overhead, the local bottleneck)
  - causal mask-mult and softmax normalization (normalize_recip) on the
    otherwise-idle GPSIMD/Pool engine; po PSUM freed by one DVE copy
  - B(0) prologue and D(3) tail draw PSUM from the double-buffered pS
    pool instead of the single-buffered filler pool
  - DMA priority order: wq+x(sc0) interleaved, then wk, wv, rest
See kernel_v2.py docstring for the algorithm itself.
"""

from contextlib import ExitStack

import numpy as np

import concourse.bass as bass
import concourse.bacc as bacc
import concourse.mybir as mybir
import concourse.tile as tile

F32 = mybir.dt.float32
BF16 = mybir.dt.bfloat16
ADD = mybir.AluOpType.add
MULT = mybir.AluOpType.mult
EXP = mybir.ActivationFunctionType.Exp

S = 2048
D = 1024
JC = 512
DK = 64
NH = 8
NSC = 4
NST = 16
ND = 8
NJ = 4
VW = 65


def build_mha():
    nc = bacc.Bacc("TRN2", target_bir_lowering=False, debug=False)

    xT = nc.dram_tensor("xT", [D, S], BF16, kind="ExternalInput").ap()
    wq = nc.dram_tensor("wq", [D, JC], BF16, kind="ExternalInput").ap()
    wk = nc.dram_tensor("wk", [D, JC], BF16, kind="ExternalInput").ap()
    wv = nc.dram_tensor("wv", [D, JC], BF16, kind="ExternalInput").ap()
    wo = nc.dram_tensor("wo", [JC, D], BF16, kind="ExternalInput").ap()
    bq = nc.dram_tensor("bq", [128, NJ], F32, kind="ExternalInput").ap()
    bk = nc.dram_tensor("bk", [128, NJ], F32, kind="ExternalInput").ap()
    bvb = nc.dram_tensor("bvb", [128, JC], F32, kind="ExternalInput").ap()
    mask = nc.dram_tensor("mask", [128, 128], BF16, kind="ExternalInput").ap()
    ident = nc.dram_tensor("ident", [128, 128], BF16, kind="ExternalInput").ap()
    out = nc.dram_tensor("out", [S, D], BF16, kind="ExternalOutput").ap()

    with tile.TileContext(nc) as tc, ExitStack() as ctx:
        const = ctx.enter_context(tc.tile_pool(name="const", bufs=1))
        bq_t = const.tile([128, NJ], F32)
        bk_t = const.tile([128, NJ], F32)
        bvb_t = const.tile([128, JC], F32)
        mask_t = const.tile([128, 128], BF16)
        ident_t = const.tile([128, 128], BF16)

        wp = ctx.enter_context(tc.tile_pool(name="wts", bufs=1))
        wq_t = wp.tile([128, ND * JC], BF16)
        wk_t = wp.tile([128, ND * JC], BF16)
        wv_t = wp.tile([128, ND * JC], BF16)
        wo_t = wp.tile([128, NJ * D], BF16)

        big = ctx.enter_context(tc.tile_pool(name="big", bufs=1))
        xT_t = big.tile([128, ND * S], BF16, tag="xT")
        qT_t = big.tile([128, NJ * S], BF16, tag="qT")
        kT_t = big.tile([128, NJ * S], BF16, tag="kT")
        v_t = big.tile([128, NST * NH * VW], BF16, tag="v")
        oT_t = big.tile([128, NJ * S], BF16, tag="oT")

        # psum: pS 3x2 + pV 2 = 8 banks (everything rotates through pS)
        pS = ctx.enter_context(tc.tile_pool(name="pS", bufs=3, space="PSUM"))
        pV = ctx.enter_context(tc.tile_pool(name="pV", bufs=2, space="PSUM"))

        ep = ctx.enter_context(tc.tile_pool(name="exp", bufs=8))
        pf = ctx.enter_context(tc.tile_pool(name="pof", bufs=4))
        ap_ = ctx.enter_context(tc.tile_pool(name="anorm", bufs=6))
        ostp = ctx.enter_context(tc.tile_pool(name="ost", bufs=6))

        # ---- DMAs: weights via SWDGE (gpsimd, idle Pool engine) in halves,
        # x via HWDGE (sync) per-chunk — two parallel DGE paths, ordered so
        # the first Q accumulation starts ~4us in.
        xT_r = xT_t[:].rearrange("p (c s) -> p c s", c=ND)
        xT_d = xT.rearrange("(c p) s -> p c s", p=128)
        wq_r = wq_t[:].rearrange("p (c j) -> p c j", c=ND)
        wk_r = wk_t[:].rearrange("p (c j) -> p c j", c=ND)
        wv_r = wv_t[:].rearrange("p (c j) -> p c j", c=ND)

        wq_d = wq.rearrange("(c p) j -> p c j", p=128)
        wk_d = wk.rearrange("(c p) j -> p c j", p=128)
        wv_d = wv.rearrange("(c p) j -> p c j", p=128)
        nc.gpsimd.dma_start(wq_r[:, 0:1], wq_d[:, 0:1])
        nc.sync.dma_start(xT_r[:, 0, 0:512], xT_d[:, 0, 0:512])
        nc.gpsimd.dma_start(wq_r[:, 1:2], wq_d[:, 1:2])
        nc.sync.dma_start(xT_r[:, 1, 0:512], xT_d[:, 1, 0:512])
        nc.gpsimd.dma_start(wq_r[:, 2:4], wq_d[:, 2:4])
        for dc in range(2, 4):
            nc.sync.dma_start(xT_r[:, dc, 0:512], xT_d[:, dc, 0:512])
        nc.gpsimd.dma_start(wq_r[:, 4:8], wq_d[:, 4:8])
        for dc in range(4, ND):
            nc.sync.dma_start(xT_r[:, dc, 0:512], xT_d[:, dc, 0:512])
        nc.sync.dma_start(bq_t[:], bq[:, :])
        nc.gpsimd.dma_start(wk_r[:, 0:4], wk_d[:, 0:4])
        nc.gpsimd.dma_start(wk_r[:, 4:8], wk_d[:, 4:8])
        nc.sync.dma_start(bk_t[:], bk[:, :])
        nc.gpsimd.dma_start(wv_r[:, 0:4], wv_d[:, 0:4])
        nc.gpsimd.dma_start(wv_r[:, 4:8], wv_d[:, 4:8])
        nc.sync.dma_start(bvb_t[:], bvb[:, :])
        nc.sync.dma_start(mask_t[:], mask[:, :])
        nc.sync.dma_start(ident_t[:], ident[:, :])
        for sc in range(1, NSC):
            nc.sync.dma_start(xT_r[:, :, sc * 512:(sc + 1) * 512],
                              xT_d[:, :, sc * 512:(sc + 1) * 512])
        nc.gpsimd.dma_start(wo_t[:].rearrange("p (c j) -> p c j", c=NJ),
                            wo.rearrange("(c p) j -> p c j", p=128))

        v_r = v_t[:].rearrange("p (st h w) -> p st h w", st=NST, h=NH)
        bvb_r = bvb_t[:].rearrange("p (h w) -> p h w", h=NH)

        # PE pstate warmup: ~3us of dummy matmuls while the first DMAs land
        with tc.tile_pool(name="warm", bufs=1) as wmp:
            wm = wmp.tile([128, 512], BF16, tag="wm", name="wm")
            nc.vector.memset(wm[:], 0.0)
            pw = pS.tile([128, 1024], F32, tag="sc", name="pw")
            for i in range(14):
                nc.tensor.matmul(pw[:, 0:512], wm[:, 0:128], wm[:],
                                 start=(i == 0), stop=(i == 13))

        # big-tile half allocator (pS): yields [128,512] f32 psum views
        _half = [None, 0]

        def ps_half():
            if _half[1] % 2 == 0:
                _half[0] = pS.tile([128, 1024], F32, tag="sc", name="psh")
            _half[1] += 1
            o = 512 * ((_half[1] - 1) % 2)
            return _half[0][:, o:o + 512]

        def ps_quads():
            t1 = pS.tile([128, 1024], F32, tag="sc", name="psq1")
            t2 = pS.tile([128, 1024], F32, tag="sc", name="psq2")
            return [t1[:, 0:512], t1[:, 512:1024],
                    t2[:, 0:512], t2[:, 512:1024]]

        # B(0) prologue: dc-outer with 4 concurrent psum groups, so compute
        # starts as soon as the first weight/x chunk lands.
        def b0_qk(w_t, b_t, dstT):
            hv = ps_quads()
            for dc in range(ND):
                for jt in range(NJ):
                    nc.tensor.matmul(
                        hv[jt],
                        w_t[:, dc * JC + jt * 128: dc * JC + (jt + 1) * 128],
                        xT_t[:, dc * S: dc * S + 512],
                        start=(dc == 0), stop=(dc == ND - 1))
            for jt in range(NJ):
                nc.vector.tensor_scalar(
                    dstT[:, jt * S: jt * S + 512],
                    hv[jt], b_t[:, jt:jt + 1], None, op0=ADD)

        def b0_v():
            for st in range(4):
                pv = ps_full()
                for dc in range(ND):
                    nc.tensor.matmul(
                        pv,
                        xT_t[:, dc * S + st * 128: dc * S + (st + 1) * 128],
                        wv_t[:, dc * JC:(dc + 1) * JC],
                        start=(dc == 0), stop=(dc == ND - 1))
                nc.vector.tensor_tensor(
                    v_r[:, st, :, 0:DK],
                    pv.rearrange("p (h w) -> p h w", h=NH), bvb_r[:],
                    op=ADD)
                nc.gpsimd.memset(v_r[:, st, :, DK:VW], 1.0)

        # ---- phase B units
        def unit_qk(w_t, b_t, dstT, sc, jt, alloc, half):
            dst = None

            def emit():
                pq = alloc()
                dcs = range(4) if half == 0 else range(4, ND)
                for dc in dcs:
                    nc.tensor.matmul(
                        pq,
                        w_t[:, dc * JC + jt * 128: dc * JC + (jt + 1) * 128],
                        xT_t[:, dc * S + sc * 512: dc * S + (sc + 1) * 512],
                        start=(dc == dcs[0]), stop=(dc == dcs[-1]))
                d = dstT[:, jt * S + sc * 512: jt * S + (sc + 1) * 512]
                if half == 0:
                    nc.vector.tensor_scalar(
                        d, pq, b_t[:, jt:jt + 1], None, op0=ADD)
                else:
                    nc.vector.tensor_tensor(d, d, pq, op=ADD)
            return emit

        def unit_v(sc, st4, alloc):
            def emit():
                st = sc * 4 + st4
                pv = alloc()
                for dc in range(ND):
                    nc.tensor.matmul(
                        pv,
                        xT_t[:, dc * S + st * 128: dc * S + (st + 1) * 128],
                        wv_t[:, dc * JC:(dc + 1) * JC],
                        start=(dc == 0), stop=(dc == ND - 1))
                nc.vector.tensor_tensor(
                    v_r[:, st, :, 0:DK],
                    pv.rearrange("p (h w) -> p h w", h=NH), bvb_r[:],
                    op=ADD)
                nc.gpsimd.memset(v_r[:, st, :, DK:VW], 1.0)
            return emit

        def b_units(sc, alloc):
            us = []
            for jt in range(NJ):
                for half in range(2):
                    us.append(unit_qk(wq_t, bq_t, qT_t, sc, jt, alloc, half))
            for jt in range(NJ):
                for half in range(2):
                    us.append(unit_qk(wk_t, bk_t, kT_t, sc, jt, alloc, half))
            for st4 in range(4):
                us.append(unit_v(sc, st4, alloc))
            return us

        # ---- phase D half-units: one (st, od) psum group each
        def d_units(sc, alloc):
            us = []
            for st in range(sc * 4, sc * 4 + 4):
                ot = [None]

                def emit(st=st, ot=ot):
                    ot[0] = ostp.tile([128, D], BF16, tag="ost", name="ot")
                    pd = alloc()
                    for vc in range(NJ):
                        nc.tensor.matmul(
                            pd,
                            oT_t[:, vc * S + st * 128: vc * S + (st + 1) * 128],
                            wo_t[:, vc * D: vc * D + 512],
                            start=(vc == 0), stop=(vc == NJ - 1))
                    nc.vector.tensor_copy(ot[0][:, 0:512], pd)
                    nc.sync.dma_start(out[st * 128:(st + 1) * 128, 0:512],
                                      ot[0][:, 0:512])

                def emit2(st=st, ot=ot):
                    pd = alloc()
                    for vc in range(NJ):
                        nc.tensor.matmul(
                            pd,
                            oT_t[:, vc * S + st * 128: vc * S + (st + 1) * 128],
                            wo_t[:, vc * D + 512: vc * D + 1024],
                            start=(vc == 0), stop=(vc == NJ - 1))
                    nc.vector.tensor_copy(ot[0][:, 512:1024], pd)
                    nc.sync.dma_start(out[st * 128:(st + 1) * 128, 512:1024],
                                      ot[0][:, 512:1024])

                us.append(emit)
                us.append(emit2)
            return us

        # ---- phase C
        pending_tr = [None]   # deferred transpose emitter (1-pair pipeline)

        def flush_tr():
            if pending_tr[0] is not None:
                pending_tr[0]()
                pending_tr[0] = None

        def phase_c(qt, units):
            n_pair = NH * (2 * qt + 2)
            frac = len(units) / n_pair if units else 0.0
            ui = iter(units)
            acc = 0.0
            a_pair = [None, None]
            po_h = {}
            av_q = []

            def make_tr(at, hp):
                # both heads' normalized outputs share one [128, 4*128]
                # tile (even head cols 0:64 of each block, odd 64:128), so
                # ONE [128,128] transpose per q-subtile covers the pair —
                # transpose cost depends only on output free size.
                def emit_tr():
                    pt = pV.tile([128, 512], BF16, tag="po",
                                 name="pt", padded_shape=[128, 1024])
                    for qsub in range(4):
                        nc.tensor.transpose(
                            pt[:, qsub * 128:(qsub + 1) * 128],
                            at[:, qsub * 128:(qsub + 1) * 128],
                            ident_t[:])
                    if qt == 3 and hp == NJ - 1:
                        for q4 in range(4):   # quartered: D(3) starts sooner
                            nc.vector.tensor_copy(
                                oT_t[:, hp * S + qt * 512 + q4 * 128:
                                     hp * S + qt * 512 + (q4 + 1) * 128],
                                pt[:, q4 * 128:(q4 + 1) * 128])
                    else:
                        nc.vector.tensor_copy(
                            oT_t[:, hp * S + qt * 512:
                                 hp * S + (qt + 1) * 512],
                            pt[:, 0:512])
                return emit_tr

            def emit_norm(h):
                hp, h2 = h // 2, h % 2
                po = po_h.pop(h)
                if h2 == 0:
                    a_pair[0] = ap_.tile([128, 4 * 128], BF16, tag="a",
                                         name="a")
                at = a_pair[0]
                if qt == 3 and h == NH - 1:
                    # tail: normalize straight from PSUM on DVE
                    po_r = po[:].rearrange("p (q w) -> p q w", q=4)
                    r = pf.tile([128, 4], F32, tag="r", name="r")
                    nc.vector.reciprocal(r[:], po_r[:, :, DK])
                    for qsub in range(4):
                        o = qsub * 128 + h2 * DK
                        nc.vector.tensor_scalar(
                            at[:, o:o + DK],
                            po_r[:, qsub, 0:DK], r[:, qsub:qsub + 1], None,
                            op0=MULT)
                else:
                    pof = pf.tile([128, 4 * VW], F32, tag="pof", name="pof")
                    nc.vector.tensor_copy(pof[:], po[:])
                    pof_r = pof[:].rearrange("p (q w) -> p q w", q=4)
                    for qsub in range(4):
                        o = qsub * 128 + h2 * DK
                        nc.gpsimd.normalize_recip(
                            at[:, o:o + DK],
                            pof_r[:, qsub, 0:DK], pof_r[:, qsub, DK:VW])
                if h2 == 1:
                    tr = make_tr(at, hp)
                    flush_tr()          # previous pair, one pair deferred
                    pending_tr[0] = tr

            for h in range(NH):
                hp, h2 = h // 2, h % 2
                lo = 64 * h2
                qcol = hp * S + qt * 512
                po = pV.tile([128, 4 * VW], F32, tag="po", name="po",
                             padded_shape=[128, 512])
                po_h[h] = po
                for kp in range(2 * qt + 2):
                    ps = pS.tile([128, 1024], F32, tag="sc", name="ps")
                    e = ep.tile([128, 1024], BF16, tag="e", name="e")
                    # column base of k2's q-range in ps/e: diagonal pairs
                    # are COMPACTED so one contiguous exp covers both
                    # halves with no junk columns.
                    dd0 = 2 * kp - 4 * qt
                    if dd0 == 0:          # diag pair 1: [0:512]+[512:896]
                        base = (0, 512 - 128)
                        erng = (0, 896)
                    elif dd0 == 2:        # diag pair 2: [256:512]+[512:640]
                        base = (0, 512 - 384)
                        erng = (256, 640)
                    else:                 # full pair
                        base = (0, 512)
                        erng = (0, 1024)
                    for k2 in range(2):
                        kt = 2 * kp + k2
                        off = max(kt - 4 * qt, 0) * 128
                        nc.tensor.matmul(
                            ps[:, base[k2] + off: base[k2] + 512],
                            kT_t[lo:lo + 64,
                                 hp * S + kt * 128: hp * S + (kt + 1) * 128],
                            qT_t[lo:lo + 64, qcol + off: qcol + 512],
                            start=True, stop=True)
                    nc.scalar.activation(e[:, erng[0]:erng[1]],
                                         ps[:, erng[0]:erng[1]],
                                         EXP, scale=0.125)
                    for k2 in range(2):
                        dd = 2 * kp + k2 - 4 * qt
                        if dd >= 0:
                            o = base[k2] + dd * 128
                            nc.gpsimd.tensor_tensor(
                                e[:, o:o + 128], e[:, o:o + 128],
                                mask_t[:], op=MULT)

                    def make_av(e=e, kp=kp, h=h, po=po, base=base):
                        def emit_av():
                            for k2 in range(2):
                                kt = 2 * kp + k2
                                dd = kt - 4 * qt
                                for qsub in range(max(dd, 0), 4):
                                    nc.tensor.matmul(
                                        po[:, qsub * VW: qsub * VW + VW],
                                        e[:, base[k2] + qsub * 128:
                                           base[k2] + (qsub + 1) * 128],
                                        v_t[:, (kt * NH + h) * VW:
                                            (kt * NH + h + 1) * VW],
                                        start=(kt == 0 and qsub == 0),
                                        stop=(kt == 4 * qt + 3 and qsub == 3),
                                        skip_group_check=True)
                        return emit_av
                    if len(av_q) >= (3 if qt >= 1 else 2):
                        av_q.pop(0)()
                    if kp == (2 if qt >= 1 else 1) and h > 0:
                        emit_norm(h - 1)
                    av_q.append(make_av())
                    acc += frac
                    while acc >= 1.0:
                        acc -= 1.0
                        u = next(ui, None)
                        if u is not None:
                            u()
            for av in av_q:
                av()
            av_q.clear()
            emit_norm(NH - 1)
            for u in ui:
                u()

        # ---- main schedule
        def ps_full():
            return pS.tile([128, 1024], F32, tag="sc", name="pp")[:, 0:512]

        b0_qk(wq_t, bq_t, qT_t)
        b0_qk(wk_t, bk_t, kT_t)
        b0_v()
        for qt in range(NSC):
            if qt < 3:
                units = b_units(qt + 1, ps_full)
            else:
                units = (d_units(0, ps_full) + d_units(1, ps_full)
                         + d_units(2, ps_full))
            phase_c(qt, units)
        flush_tr()
        for u in d_units(3, ps_full):
            u()

    nc.compile()
    return nc


# ----------------------------------------------------------------- host side

_NC_CACHE = None


def _get_nc():
    global _NC_CACHE
    if _NC_CACHE is None:
        _NC_CACHE = build_mha()
    return _NC_CACHE


def shard_inputs(x, Wq, bq, Wk, bk, Wv, bv, Wo, bo):
    import ml_dtypes
    bf16 = ml_dtypes.bfloat16
    mask = np.triu(np.ones((128, 128), np.float32)).astype(bf16)
    ident = np.eye(128, dtype=np.float32).astype(bf16)
    x = np.asarray(x, dtype=np.float32)
    xTs = [np.ascontiguousarray(x[b].T).astype(bf16) for b in range(4)]
    Wq, Wk, Wv, Wo = (np.asarray(w, np.float32) for w in (Wq, Wk, Wv, Wo))
    bq, bk, bv = (np.asarray(b_, np.float32) for b_ in (bq, bk, bv))
    maps = []
    for c in range(8):
        b, g = c // 2, c % 2
        sl = slice(g * JC, (g + 1) * JC)
        maps.append({
            "xT": xTs[b],
            "wq": np.ascontiguousarray(Wq[:, sl]).astype(bf16),
            "wk": np.ascontiguousarray(Wk[:, sl]).astype(bf16),
            "wv": np.ascontiguousarray(Wv[:, sl]).astype(bf16),
            "wo": np.ascontiguousarray(Wo[sl, :]).astype(bf16),
            "bq": np.ascontiguousarray(bq[sl].reshape(NJ, 128).T),
            "bk": np.ascontiguousarray(bk[sl].reshape(NJ, 128).T),
            "bvb": np.broadcast_to(bv[sl], (128, JC)).copy(),
            "mask": mask,
            "ident": ident,
        })
    return maps


def kernel(x, Wq, bq, Wk, bk, Wv, bv, Wo, bo):
    """Full-input entry point: shard across 8 NeuronCores, run, gather."""
    from concourse.bass_utils import run_bass_kernel_spmd

    nc = _get_nc()
    in_maps = shard_inputs(x, Wq, bq, Wk, bk, Wv, bv, Wo, bo)
    res = run_bass_kernel_spmd(nc, in_maps, list(range(8)))
    bo = np.asarray(bo, dtype=np.float32)
    out = np.empty((4, S, D), dtype=np.float32)
    for b in range(4):
        out[b] = (res.results[2 * b]["out"].astype(np.float32)
                  + res.results[2 * b + 1]["out"].astype(np.float32) + bo)
    return out


# revision 8
# speedup vs baseline: 1.0613x; 1.0162x over previous
"""Bass/Tile MHA kernel for TRN2 — per-core program, v3.

v2 + scheduling/engine-balance work:
  - scores k-tiles paired into [128,1024] PSUM tiles; one exp per pair
    (halves Act per-instruction overhead, the local bottleneck)
  - causal mask-mult and softmax normalization (normalize_recip) on the
    otherwise-idle GPSIMD/Pool engine; po PSUM freed by one DVE copy
  - B(0) prologue and D(3) tail draw PSUM from the double-buffered pS
    pool instead of the single-buffered filler pool
  - DMA priority order: wq+x(sc0) interleaved, then wk, wv, rest
See kernel_v2.py docstring for the algorithm itself.
"""

from contextlib import ExitStack

import numpy as np

import concourse.bass as bass
import concourse.bacc as bacc
import concourse.mybir as mybir
import concourse.tile as tile

F32 = mybir.dt.float32
BF16 = mybir.dt.bfloat16
ADD = mybir.AluOpType.add
MULT = mybir.AluOpType.mult
EXP = mybir.ActivationFunctionType.Exp

S = 2048
D = 1024
JC = 512
DK = 64
NH = 8
NSC = 4
NST = 16
ND = 8
NJ = 4
VW = 65


def build_mha():
    nc = bacc.Bacc("TRN2", target_bir_lowering=False, debug=False)

    xT = nc.dram_tensor("xT", [D, S], BF16, kind="ExternalInput").ap()
    wq = nc.dram_tensor("wq", [D, JC], BF16, kind="ExternalInput").ap()
    wk = nc.dram_tensor("wk", [D, JC], BF16, kind="ExternalInput").ap()
    wv = nc.dram_tensor("wv", [D, JC], BF16, kind="ExternalInput").ap()
    wo = nc.dram_tensor("wo", [JC, D], BF16, kind="ExternalInput").ap()
    bq = nc.dram_tensor("bq", [128, NJ], F32, kind="ExternalInput").ap()
    bk = nc.dram_tensor("bk", [128, NJ], F32, kind="ExternalInput").ap()
    bvb = nc.dram_tensor("bvb", [128, JC], F32, kind="ExternalInput").ap()
    mask = nc.dram_tensor("mask", [128, 128], BF16, kind="ExternalInput").ap()
    ident = nc.dram_tensor("ident", [128, 128], BF16, kind="ExternalInput").ap()
    out = nc.dram_tensor("out", [S, D], BF16, kind="ExternalOutput").ap()

    with tile.TileContext(nc) as tc, ExitStack() as ctx:
        const = ctx.enter_context(tc.tile_pool(name="const", bufs=1))
        bq_t = const.tile([128, NJ], F32)
        bk_t = const.tile([128, NJ], F32)
        bvb_t = const.tile([128, JC], F32)
        mask_t = const.tile([128, 128], BF16)
        ident_t = const.tile([128, 128], BF16)

        wp = ctx.enter_context(tc.tile_pool(name="wts", bufs=1))
        wq_t = wp.tile([128, ND * JC], BF16)
        wk_t = wp.tile([128, ND * JC], BF16)
        wv_t = wp.tile([128, ND * JC], BF16)
        wo_t = wp.tile([128, NJ * D], BF16)

        big = ctx.enter_context(tc.tile_pool(name="big", bufs=1))
        xT_t = big.tile([128, ND * S], BF16, tag="xT")
        qT_t = big.tile([128, NJ * S], BF16, tag="qT")
        kT_t = big.tile([128, NJ * S], BF16, tag="kT")
        v_t = big.tile([128, NST * NH * VW], BF16, tag="v")
        oT_t = big.tile([128, NJ * S], BF16, tag="oT")

        # psum: pS 3x2 + pV 2 = 8 banks (everything rotates through pS)
        pS = ctx.enter_context(tc.tile_pool(name="pS", bufs=3, space="PSUM"))
        pV = ctx.enter_context(tc.tile_pool(name="pV", bufs=2, space="PSUM"))

        ep = ctx.enter_context(tc.tile_pool(name="exp", bufs=8))
        pf = ctx.enter_context(tc.tile_pool(name="pof", bufs=4))
        ap_ = ctx.enter_context(tc.tile_pool(name="anorm", bufs=6))
        ostp = ctx.enter_context(tc.tile_pool(name="ost", bufs=6))

        # ---- DMAs: weights via SWDGE (gpsimd, idle Pool engine) in halves,
        # x via HWDGE (sync) per-chunk — two parallel DGE paths, ordered so
        # the first Q accumulation starts ~4us in.
        xT_r = xT_t[:].rearrange("p (c s) -> p c s", c=ND)
        xT_d = xT.rearrange("(c p) s -> p c s", p=128)
        wq_r = wq_t[:].rearrange("p (c j) -> p c j", c=ND)
        wk_r = wk_t[:].rearrange("p (c j) -> p c j", c=ND)
        wv_r = wv_t[:].rearrange("p (c j) -> p c j", c=ND)

        wq_d = wq.rearrange("(c p) j -> p c j", p=128)
        wk_d = wk.rearrange("(c p) j -> p c j", p=128)
        wv_d = wv.rearrange("(c p) j -> p c j", p=128)
        nc.gpsimd.dma_start(wq_r[:, 0:1], wq_d[:, 0:1])
        nc.sync.dma_start(xT_r[:, 0, 0:512], xT_d[:, 0, 0:512])
        nc.gpsimd.dma_start(wq_r[:, 1:2], wq_d[:, 1:2])
        nc.sync.dma_start(xT_r[:, 1, 0:512], xT_d[:, 1, 0:512])
        nc.gpsimd.dma_start(wq_r[:, 2:4], wq_d[:, 2:4])
        for dc in range(2, 4):
            nc.sync.dma_start(xT_r[:, dc, 0:512], xT_d[:, dc, 0:512])
        nc.gpsimd.dma_start(wq_r[:, 4:8], wq_d[:, 4:8])
        for dc in range(4, ND):
            nc.sync.dma_start(xT_r[:, dc, 0:512], xT_d[:, dc, 0:512])
        nc.sync.dma_start(bq_t[:], bq[:, :])
        nc.gpsimd.dma_start(wk_r[:, 0:4], wk_d[:, 0:4])
        nc.gpsimd.dma_start(wk_r[:, 4:8], wk_d[:, 4:8])
        nc.sync.dma_start(bk_t[:], bk[:, :])
        nc.gpsimd.dma_start(wv_r[:, 0:4], wv_d[:, 0:4])
        nc.gpsimd.dma_start(wv_r[:, 4:8], wv_d[:, 4:8])
        nc.sync.dma_start(bvb_t[:], bvb[:, :])
        nc.sync.dma_start(mask_t[:], mask[:, :])
        nc.sync.dma_start(ident_t[:], ident[:, :])
        for sc in range(1, NSC):
            nc.sync.dma_start(xT_r[:, :, sc * 512:(sc + 1) * 512],
                              xT_d[:, :, sc * 512:(sc + 1) * 512])
        nc.gpsimd.dma_start(wo_t[:].rearrange("p (c j) -> p c j", c=NJ),
                            wo.rearrange("(c p) j -> p c j", p=128))

        v_r = v_t[:].rearrange("p (st h w) -> p st h w", st=NST, h=NH)
        bvb_r = bvb_t[:].rearrange("p (h w) -> p h w", h=NH)

        # PE pstate warmup: ~3us of dummy matmuls while the first DMAs land
        with tc.tile_pool(name="warm", bufs=1) as wmp:
            wm = wmp.tile([128, 512], BF16, tag="wm", name="wm")
            nc.vector.memset(wm[:], 0.0)
            pw = pS.tile([128, 1024], F32, tag="sc", name="pw")
            for i in range(14):
                nc.tensor.matmul(pw[:, 0:512], wm[:, 0:128], wm[:],
                                 start=(i == 0), stop=(i == 13))

        # big-tile half allocator (pS): yields [128,512] f32 psum views
        _half = [None, 0]

        def ps_half():
            if _half[1] % 2 == 0:
                _half[0] = pS.tile([128, 1024], F32, tag="sc", name="psh")
            _half[1] += 1
            o = 512 * ((_half[1] - 1) % 2)
            return _half[0][:, o:o + 512]

        def ps_quads():
            t1 = pS.tile([128, 1024], F32, tag="sc", name="psq1")
            t2 = pS.tile([128, 1024], F32, tag="sc", name="psq2")
            return [t1[:, 0:512], t1[:, 512:1024],
                    t2[:, 0:512], t2[:, 512:1024]]

        # B(0) prologue: dc-outer with 4 concurrent psum groups, so compute
        # starts as soon as the first weight/x chunk lands.
        def b0_qk(w_t, b_t, dstT):
            hv = ps_quads()
            for dc in range(ND):
                for jt in range(NJ):
                    nc.tensor.matmul(
                        hv[jt],
                        w_t[:, dc * JC + jt * 128: dc * JC + (jt + 1) * 128],
                        xT_t[:, dc * S: dc * S + 512],
                        start=(dc == 0), stop=(dc == ND - 1))
            for jt in range(NJ):
                # Act is idle during B(0); keep DVE clear for C(0)'s start
                nc.scalar.activation(
                    dstT[:, jt * S: jt * S + 512], hv[jt],
                    mybir.ActivationFunctionType.Identity,
                    bias=b_t[:, jt:jt + 1])

        def b0_v():
            for st in range(4):
                pv = ps_full()
                for dc in range(ND):
                    nc.tensor.matmul(
                        pv,
                        xT_t[:, dc * S + st * 128: dc * S + (st + 1) * 128],
                        wv_t[:, dc * JC:(dc + 1) * JC],
                        start=(dc == 0), stop=(dc == ND - 1))
                nc.vector.tensor_tensor(
                    v_r[:, st, :, 0:DK],
                    pv.rearrange("p (h w) -> p h w", h=NH), bvb_r[:],
                    op=ADD)
                nc.gpsimd.memset(v_r[:, st, :, DK:VW], 1.0)

        # ---- phase B units
        def unit_qk(w_t, b_t, dstT, sc, jt, alloc, half):
            dst = None

            def emit():
                pq = alloc()
                dcs = range(4) if half == 0 else range(4, ND)
                for dc in dcs:
                    nc.tensor.matmul(
                        pq,
                        w_t[:, dc * JC + jt * 128: dc * JC + (jt + 1) * 128],
                        xT_t[:, dc * S + sc * 512: dc * S + (sc + 1) * 512],
                        start=(dc == dcs[0]), stop=(dc == dcs[-1]))
                d = dstT[:, jt * S + sc * 512: jt * S + (sc + 1) * 512]
                if half == 0:
                    nc.vector.tensor_scalar(
                        d, pq, b_t[:, jt:jt + 1], None, op0=ADD)
                else:
                    nc.vector.tensor_tensor(d, d, pq, op=ADD)
            return emit

        def unit_v(sc, st4, alloc):
            def emit():
                st = sc * 4 + st4
                pv = alloc()
                for dc in range(ND):
                    nc.tensor.matmul(
                        pv,
                        xT_t[:, dc * S + st * 128: dc * S + (st + 1) * 128],
                        wv_t[:, dc * JC:(dc + 1) * JC],
                        start=(dc == 0), stop=(dc == ND - 1))
                nc.vector.tensor_tensor(
                    v_r[:, st, :, 0:DK],
                    pv.rearrange("p (h w) -> p h w", h=NH), bvb_r[:],
                    op=ADD)
                nc.gpsimd.memset(v_r[:, st, :, DK:VW], 1.0)
            return emit

        def b_units(sc, alloc):
            us = []
            for jt in range(NJ):
                for half in range(2):
                    us.append(unit_qk(wq_t, bq_t, qT_t, sc, jt, alloc, half))
            for jt in range(NJ):
                for half in range(2):
                    us.append(unit_qk(wk_t, bk_t, kT_t, sc, jt, alloc, half))
            for st4 in range(4):
                us.append(unit_v(sc, st4, alloc))
            return us

        # ---- phase D half-units: one (st, od) psum group each
        def d_units(sc, alloc):
            us = []
            for st in range(sc * 4, sc * 4 + 4):
                ot = [None]

                def emit(st=st, ot=ot):
                    ot[0] = ostp.tile([128, D], BF16, tag="ost", name="ot")
                    pd = alloc()
                    for vc in range(NJ):
                        nc.tensor.matmul(
                            pd,
                            oT_t[:, vc * S + st * 128: vc * S + (st + 1) * 128],
                            wo_t[:, vc * D: vc * D + 512],
                            start=(vc == 0), stop=(vc == NJ - 1))
                    nc.vector.tensor_copy(ot[0][:, 0:512], pd)
                    nc.sync.dma_start(out[st * 128:(st + 1) * 128, 0:512],
                                      ot[0][:, 0:512])

                def emit2(st=st, ot=ot):
                    pd = alloc()
                    for vc in range(NJ):
                        nc.tensor.matmul(
                            pd,
                            oT_t[:, vc * S + st * 128: vc * S + (st + 1) * 128],
                            wo_t[:, vc * D + 512: vc * D + 1024],
                            start=(vc == 0), stop=(vc == NJ - 1))
                    nc.vector.tensor_copy(ot[0][:, 512:1024], pd)
                    nc.sync.dma_start(out[st * 128:(st + 1) * 128, 512:1024],
                                      ot[0][:, 512:1024])

                us.append(emit)
                us.append(emit2)
            return us

        # ---- phase C
        pending_tr = [None]   # deferred transpose emitter (1-pair pipeline)

        def flush_tr():
            if pending_tr[0] is not None:
                pending_tr[0]()
                pending_tr[0] = None

        def phase_c(qt, units):
            n_pair = NH * (2 * qt + 2)
            frac = len(units) / n_pair if units else 0.0
            ui = iter(units)
            acc = 0.0
            a_pair = [None, None]
            po_h = {}
            av_q = []

            def make_tr(at, hp):
                # both heads' normalized outputs share one [128, 4*128]
                # tile (even head cols 0:64 of each block, odd 64:128), so
                # ONE [128,128] transpose per q-subtile covers the pair —
                # transpose cost depends only on output free size.
                def emit_tr():
                    pt = pV.tile([128, 512], BF16, tag="po",
                                 name="pt", padded_shape=[128, 1024])
                    for qsub in range(4):
                        nc.tensor.transpose(
                            pt[:, qsub * 128:(qsub + 1) * 128],
                            at[:, qsub * 128:(qsub + 1) * 128],
                            ident_t[:])
                    if qt == 3 and hp == NJ - 1:
                        for q4 in range(4):   # quartered: D(3) starts sooner
                            nc.vector.tensor_copy(
                                oT_t[:, hp * S + qt * 512 + q4 * 128:
                                     hp * S + qt * 512 + (q4 + 1) * 128],
                                pt[:, q4 * 128:(q4 + 1) * 128])
                    else:
                        nc.vector.tensor_copy(
                            oT_t[:, hp * S + qt * 512:
                                 hp * S + (qt + 1) * 512],
                            pt[:, 0:512])
                return emit_tr

            def emit_norm(h):
                hp, h2 = h // 2, h % 2
                po = po_h.pop(h)
                if h2 == 0:
                    a_pair[0] = ap_.tile([128, 4 * 128], BF16, tag="a",
                                         name="a")
                at = a_pair[0]
                if qt == 3 and h == NH - 1:
                    # tail: normalize straight from PSUM on DVE
                    po_r = po[:].rearrange("p (q w) -> p q w", q=4)
                    r = pf.tile([128, 4], F32, tag="r", name="r")
                    nc.vector.reciprocal(r[:], po_r[:, :, DK])
                    for qsub in range(4):
                        o = qsub * 128 + h2 * DK
                        nc.vector.tensor_scalar(
                            at[:, o:o + DK],
                            po_r[:, qsub, 0:DK], r[:, qsub:qsub + 1], None,
                            op0=MULT)
                else:
                    pof = pf.tile([128, 4 * VW], F32, tag="pof", name="pof")
                    nc.vector.tensor_copy(pof[:], po[:])
                    pof_r = pof[:].rearrange("p (q w) -> p q w", q=4)
                    for qsub in range(4):
                        o = qsub * 128 + h2 * DK
                        nc.gpsimd.normalize_recip(
                            at[:, o:o + DK],
                            pof_r[:, qsub, 0:DK], pof_r[:, qsub, DK:VW])
                if h2 == 1:
                    tr = make_tr(at, hp)
                    flush_tr()          # previous pair, one pair deferred
                    pending_tr[0] = tr

            for h in range(NH):
                hp, h2 = h // 2, h % 2
                lo = 64 * h2
                qcol = hp * S + qt * 512
                po = pV.tile([128, 4 * VW], F32, tag="po", name="po",
                             padded_shape=[128, 512])
                po_h[h] = po
                for kp in range(2 * qt + 2):
                    ps = pS.tile([128, 1024], F32, tag="sc", name="ps")
                    e = ep.tile([128, 1024], BF16, tag="e", name="e")
                    # column base of k2's q-range in ps/e: diagonal pairs
                    # are COMPACTED so one contiguous exp covers both
                    # halves with no junk columns.
                    dd0 = 2 * kp - 4 * qt
                    if dd0 == 0:          # diag pair 1: [0:512]+[512:896]
                        base = (0, 512 - 128)
                        erng = (0, 896)
                    elif dd0 == 2:        # diag pair 2: [256:512]+[512:640]
                        base = (0, 512 - 384)
                        erng = (256, 640)
                    else:                 # full pair
                        base = (0, 512)
                        erng = (0, 1024)
                    for k2 in range(2):
                        kt = 2 * kp + k2
                        off = max(kt - 4 * qt, 0) * 128
                        nc.tensor.matmul(
                            ps[:, base[k2] + off: base[k2] + 512],
                            kT_t[lo:lo + 64,
                                 hp * S + kt * 128: hp * S + (kt + 1) * 128],
                            qT_t[lo:lo + 64, qcol + off: qcol + 512],
                            start=True, stop=True)
                    nc.scalar.activation(e[:, erng[0]:erng[1]],
                                         ps[:, erng[0]:erng[1]],
                                         EXP, scale=0.125)
                    for k2 in range(2):
                        dd = 2 * kp + k2 - 4 * qt
                        if dd >= 0:
                            o = base[k2] + dd * 128
                            nc.gpsimd.tensor_tensor(
                                e[:, o:o + 128], e[:, o:o + 128],
                                mask_t[:], op=MULT)

                    def make_av(e=e, kp=kp, h=h, po=po, base=base):
                        def emit_av():
                            for k2 in range(2):
                                kt = 2 * kp + k2
                                dd = kt - 4 * qt
                                for qsub in range(max(dd, 0), 4):
                                    nc.tensor.matmul(
                                        po[:, qsub * VW: qsub * VW + VW],
                                        e[:, base[k2] + qsub * 128:
                                           base[k2] + (qsub + 1) * 128],
                                        v_t[:, (kt * NH + h) * VW:
                                            (kt * NH + h + 1) * VW],
                                        start=(kt == 0 and qsub == 0),
                                        stop=(kt == 4 * qt + 3 and qsub == 3),
                                        skip_group_check=True)
                        return emit_av
                    if len(av_q) >= (3 if qt >= 1 else 2):
                        av_q.pop(0)()
                    if kp == (2 if qt >= 1 else 1) and h > 0:
                        emit_norm(h - 1)
                    av_q.append(make_av())
                    acc += frac
                    while acc >= 1.0:
                        acc -= 1.0
                        u = next(ui, None)
                        if u is not None:
                            u()
            for av in av_q:
                av()
            av_q.clear()
            emit_norm(NH - 1)
            for u in ui:
                u()

        # ---- main schedule
        def ps_full():
            return pS.tile([128, 1024], F32, tag="sc", name="pp")[:, 0:512]

        b0_qk(wq_t, bq_t, qT_t)
        b0_qk(wk_t, bk_t, kT_t)
        b0_v()
        for qt in range(NSC):
            if qt < 3:
                units = b_units(qt + 1, ps_full)
            else:
                units = (d_units(0, ps_full) + d_units(1, ps_full)
                         + d_units(2, ps_full))
            phase_c(qt, units)
        flush_tr()
        for u in d_units(3, ps_full):
            u()

    nc.compile()
    return nc


# ----------------------------------------------------------------- host side

_NC_CACHE = None


def _get_nc():
    global _NC_CACHE
    if _NC_CACHE is None:
        _NC_CACHE = build_mha()
    return _NC_CACHE


def shard_inputs(x, Wq, bq, Wk, bk, Wv, bv, Wo, bo):
    import ml_dtypes
    bf16 = ml_dtypes.bfloat16
    mask = np.triu(np.ones((128, 128), np.float32)).astype(bf16)
    ident = np.eye(128, dtype=np.float32).astype(bf16)
    x = np.asarray(x, dtype=np.float32)
    xTs = [np.ascontiguousarray(x[b].T).astype(bf16) for b in range(4)]
    Wq, Wk, Wv, Wo = (np.asarray(w, np.float32) for w in (Wq, Wk, Wv, Wo))
    bq, bk, bv = (np.asarray(b_, np.float32) for b_ in (bq, bk, bv))
    maps = []
    for c in range(8):
        b, g = c // 2, c % 2
        sl = slice(g * JC, (g + 1) * JC)
        maps.append({
            "xT": xTs[b],
            "wq": np.ascontiguousarray(Wq[:, sl]).astype(bf16),
            "wk": np.ascontiguousarray(Wk[:, sl]).astype(bf16),
            "wv": np.ascontiguousarray(Wv[:, sl]).astype(bf16),
            "wo": np.ascontiguousarray(Wo[sl, :]).astype(bf16),
            "bq": np.ascontiguousarray(bq[sl].reshape(NJ, 128).T),
            "bk": np.ascontiguousarray(bk[sl].reshape(NJ, 128).T),
            "bvb": np.broadcast_to(bv[sl], (128, JC)).copy(),
            "mask": mask,
            "ident": ident,
        })
    return maps


def kernel(x, Wq, bq, Wk, bk, Wv, bv, Wo, bo):
    """Full-input entry point: shard across 8 NeuronCores, run, gather."""
    from concourse.bass_utils import run_bass_kernel_spmd

    nc = _get_nc()
    in_maps = shard_inputs(x, Wq, bq, Wk, bk, Wv, bv, Wo, bo)
    res = run_bass_kernel_spmd(nc, in_maps, list(range(8)))
    bo = np.asarray(bo, dtype=np.float32)
    out = np.empty((4, S, D), dtype=np.float32)
    for b in range(4):
        out[b] = (res.results[2 * b]["out"].astype(np.float32)
                  + res.results[2 * b + 1]["out"].astype(np.float32) + bo)
    return out


# revision 10
# speedup vs baseline: 1.0687x; 1.0070x over previous
"""Bass/Tile MHA kernel for TRN2 — per-core program, v3.

v2 + scheduling/engine-balance work:
  - scores k-tiles paired into [128,1024] PSUM tiles; one exp per pair
    (halves Act per-instruction overhead, the local bottleneck)
  - causal mask-mult and softmax normalization (normalize_recip) on the
    otherwise-idle GPSIMD/Pool engine; po PSUM freed by one DVE copy
  - B(0) prologue and D(3) tail draw PSUM from the double-buffered pS
    pool instead of the single-buffered filler pool
  - DMA priority order: wq+x(sc0) interleaved, then wk, wv, rest
See kernel_v2.py docstring for the algorithm itself.
"""

from contextlib import ExitStack

import numpy as np

import concourse.bass as bass
import concourse.bacc as bacc
import concourse.mybir as mybir
import concourse.tile as tile

F32 = mybir.dt.float32
BF16 = mybir.dt.bfloat16
ADD = mybir.AluOpType.add
MULT = mybir.AluOpType.mult
EXP = mybir.ActivationFunctionType.Exp

S = 2048
D = 1024
JC = 512
DK = 64
NH = 8
NSC = 4
NST = 16
ND = 8
NJ = 4
VW = 65


def build_mha():
    nc = bacc.Bacc("TRN2", target_bir_lowering=False, debug=False)

    xT = nc.dram_tensor("xT", [D, S], BF16, kind="ExternalInput").ap()
    wq = nc.dram_tensor("wq", [D, JC], BF16, kind="ExternalInput").ap()
    wk = nc.dram_tensor("wk", [D, JC], BF16, kind="ExternalInput").ap()
    wv = nc.dram_tensor("wv", [D, JC], BF16, kind="ExternalInput").ap()
    wo = nc.dram_tensor("wo", [JC, D], BF16, kind="ExternalInput").ap()
    bq = nc.dram_tensor("bq", [128, NJ], F32, kind="ExternalInput").ap()
    bk = nc.dram_tensor("bk", [128, NJ], F32, kind="ExternalInput").ap()
    bvb = nc.dram_tensor("bvb", [128, JC], F32, kind="ExternalInput").ap()
    mask = nc.dram_tensor("mask", [128, 128], BF16, kind="ExternalInput").ap()
    ident = nc.dram_tensor("ident", [128, 128], BF16, kind="ExternalInput").ap()
    out = nc.dram_tensor("out", [S, D], BF16, kind="ExternalOutput").ap()

    with tile.TileContext(nc) as tc, ExitStack() as ctx:
        const = ctx.enter_context(tc.tile_pool(name="const", bufs=1))
        bq_t = const.tile([128, NJ], F32)
        bk_t = const.tile([128, NJ], F32)
        bvb_t = const.tile([128, JC], F32)
        mask_t = const.tile([128, 128], BF16)
        ident_t = const.tile([128, 128], BF16)

        wp = ctx.enter_context(tc.tile_pool(name="wts", bufs=1))
        wq_t = wp.tile([128, ND * JC], BF16)
        wk_t = wp.tile([128, ND * JC], BF16)
        wv_t = wp.tile([128, ND * JC], BF16)
        wo_t = wp.tile([128, NJ * D], BF16)

        big = ctx.enter_context(tc.tile_pool(name="big", bufs=1))
        xT_t = big.tile([128, ND * S], BF16, tag="xT")
        qT_t = big.tile([128, NJ * S], BF16, tag="qT")
        kT_t = big.tile([128, NJ * S], BF16, tag="kT")
        v_t = big.tile([128, NST * NH * VW], BF16, tag="v")
        oT_t = big.tile([128, NJ * S], BF16, tag="oT")

        # psum: pS 3x2 + pV 2 = 8 banks (everything rotates through pS)
        pS = ctx.enter_context(tc.tile_pool(name="pS", bufs=3, space="PSUM"))
        pV = ctx.enter_context(tc.tile_pool(name="pV", bufs=2, space="PSUM"))

        ep = ctx.enter_context(tc.tile_pool(name="exp", bufs=8))
        pf = ctx.enter_context(tc.tile_pool(name="pof", bufs=4))
        ap_ = ctx.enter_context(tc.tile_pool(name="anorm", bufs=6))
        ostp = ctx.enter_context(tc.tile_pool(name="ost", bufs=6))

        # ---- DMAs: weights via SWDGE (gpsimd, idle Pool engine) in halves,
        # x via HWDGE (sync) per-chunk — two parallel DGE paths, ordered so
        # the first Q accumulation starts ~4us in.
        xT_r = xT_t[:].rearrange("p (c s) -> p c s", c=ND)
        xT_d = xT.rearrange("(c p) s -> p c s", p=128)
        wq_r = wq_t[:].rearrange("p (c j) -> p c j", c=ND)
        wk_r = wk_t[:].rearrange("p (c j) -> p c j", c=ND)
        wv_r = wv_t[:].rearrange("p (c j) -> p c j", c=ND)

        wq_d = wq.rearrange("(c p) j -> p c j", p=128)
        wk_d = wk.rearrange("(c p) j -> p c j", p=128)
        wv_d = wv.rearrange("(c p) j -> p c j", p=128)
        nc.gpsimd.dma_start(wq_r[:, 0:1], wq_d[:, 0:1])
        nc.sync.dma_start(xT_r[:, 0, 0:512], xT_d[:, 0, 0:512])
        nc.gpsimd.dma_start(wq_r[:, 1:2], wq_d[:, 1:2])
        nc.sync.dma_start(xT_r[:, 1, 0:512], xT_d[:, 1, 0:512])
        nc.gpsimd.dma_start(wq_r[:, 2:4], wq_d[:, 2:4])
        for dc in range(2, 4):
            nc.sync.dma_start(xT_r[:, dc, 0:512], xT_d[:, dc, 0:512])
        nc.gpsimd.dma_start(wq_r[:, 4:8], wq_d[:, 4:8])
        for dc in range(4, ND):
            nc.sync.dma_start(xT_r[:, dc, 0:512], xT_d[:, dc, 0:512])
        nc.sync.dma_start(bq_t[:], bq[:, :])
        nc.gpsimd.dma_start(wk_r[:, 0:4], wk_d[:, 0:4])
        nc.gpsimd.dma_start(wk_r[:, 4:8], wk_d[:, 4:8])
        nc.sync.dma_start(bk_t[:], bk[:, :])
        nc.gpsimd.dma_start(wv_r[:, 0:4], wv_d[:, 0:4])
        nc.gpsimd.dma_start(wv_r[:, 4:8], wv_d[:, 4:8])
        nc.sync.dma_start(bvb_t[:], bvb[:, :])
        nc.sync.dma_start(mask_t[:], mask[:, :])
        nc.sync.dma_start(ident_t[:], ident[:, :])
        for sc in range(1, NSC):
            nc.sync.dma_start(xT_r[:, :, sc * 512:(sc + 1) * 512],
                              xT_d[:, :, sc * 512:(sc + 1) * 512])
        nc.gpsimd.dma_start(wo_t[:].rearrange("p (c j) -> p c j", c=NJ),
                            wo.rearrange("(c p) j -> p c j", p=128))

        v_r = v_t[:].rearrange("p (st h w) -> p st h w", st=NST, h=NH)
        bvb_r = bvb_t[:].rearrange("p (h w) -> p h w", h=NH)

        # PE pstate warmup: ~3us of dummy matmuls while the first DMAs land
        with tc.tile_pool(name="warm", bufs=1) as wmp:
            wm = wmp.tile([128, 128], BF16, tag="wm", name="wm")
            nc.vector.memset(wm[:], 0.0)
            pw = pS.tile([128, 1024], F32, tag="sc", name="pw")
            for i in range(24):
                nc.tensor.matmul(pw[:, 0:128], wm[:], wm[:],
                                 start=(i == 0), stop=(i == 23))

        # big-tile half allocator (pS): yields [128,512] f32 psum views
        _half = [None, 0]

        def ps_half():
            if _half[1] % 2 == 0:
                _half[0] = pS.tile([128, 1024], F32, tag="sc", name="psh")
            _half[1] += 1
            o = 512 * ((_half[1] - 1) % 2)
            return _half[0][:, o:o + 512]

        def ps_quads():
            t1 = pS.tile([128, 1024], F32, tag="sc", name="psq1")
            t2 = pS.tile([128, 1024], F32, tag="sc", name="psq2")
            return [t1[:, 0:512], t1[:, 512:1024],
                    t2[:, 0:512], t2[:, 512:1024]]

        # B(0) prologue: dc-outer with 4 concurrent psum groups, so compute
        # starts as soon as the first weight/x chunk lands.
        def b0_qk(w_t, b_t, dstT):
            hv = ps_quads()
            for dc in range(ND):
                for jt in range(NJ):
                    nc.tensor.matmul(
                        hv[jt],
                        w_t[:, dc * JC + jt * 128: dc * JC + (jt + 1) * 128],
                        xT_t[:, dc * S: dc * S + 512],
                        start=(dc == 0), stop=(dc == ND - 1))
            for jt in range(NJ):
                # Act is idle during B(0); keep DVE clear for C(0)'s start
                nc.scalar.activation(
                    dstT[:, jt * S: jt * S + 512], hv[jt],
                    mybir.ActivationFunctionType.Identity,
                    bias=b_t[:, jt:jt + 1])

        def b0_v():
            for st in range(4):
                pv = ps_full()
                for dc in range(ND):
                    nc.tensor.matmul(
                        pv,
                        xT_t[:, dc * S + st * 128: dc * S + (st + 1) * 128],
                        wv_t[:, dc * JC:(dc + 1) * JC],
                        start=(dc == 0), stop=(dc == ND - 1))
                nc.vector.tensor_tensor(
                    v_r[:, st, :, 0:DK],
                    pv.rearrange("p (h w) -> p h w", h=NH), bvb_r[:],
                    op=ADD)
                nc.gpsimd.memset(v_r[:, st, :, DK:VW], 1.0)

        # ---- phase B units
        def unit_qk(w_t, b_t, dstT, sc, jt, alloc, half):
            dst = None

            def emit():
                pq = alloc()
                dcs = range(4) if half == 0 else range(4, ND)
                for dc in dcs:
                    nc.tensor.matmul(
                        pq,
                        w_t[:, dc * JC + jt * 128: dc * JC + (jt + 1) * 128],
                        xT_t[:, dc * S + sc * 512: dc * S + (sc + 1) * 512],
                        start=(dc == dcs[0]), stop=(dc == dcs[-1]))
                d = dstT[:, jt * S + sc * 512: jt * S + (sc + 1) * 512]
                if half == 0:
                    nc.vector.tensor_scalar(
                        d, pq, b_t[:, jt:jt + 1], None, op0=ADD)
                else:
                    nc.vector.tensor_tensor(d, d, pq, op=ADD)
            return emit

        def unit_v(sc, st4, alloc):
            def emit():
                st = sc * 4 + st4
                pv = alloc()
                for dc in range(ND):
                    nc.tensor.matmul(
                        pv,
                        xT_t[:, dc * S + st * 128: dc * S + (st + 1) * 128],
                        wv_t[:, dc * JC:(dc + 1) * JC],
                        start=(dc == 0), stop=(dc == ND - 1))
                nc.vector.tensor_tensor(
                    v_r[:, st, :, 0:DK],
                    pv.rearrange("p (h w) -> p h w", h=NH), bvb_r[:],
                    op=ADD)
                nc.gpsimd.memset(v_r[:, st, :, DK:VW], 1.0)
            return emit

        def b_units(sc, alloc):
            us = []
            for jt in range(NJ):
                for half in range(2):
                    us.append(unit_qk(wq_t, bq_t, qT_t, sc, jt, alloc, half))
            for jt in range(NJ):
                for half in range(2):
                    us.append(unit_qk(wk_t, bk_t, kT_t, sc, jt, alloc, half))
            for st4 in range(4):
                us.append(unit_v(sc, st4, alloc))
            return us

        # ---- phase D half-units: one (st, od) psum group each
        def d_units(sc, alloc, cp=None):
            us = []
            for st in range(sc * 4, sc * 4 + 4):
                ot = [None]

                def emit(st=st, ot=ot):
                    ot[0] = ostp.tile([128, D], BF16, tag="ost", name="ot")
                    pd = alloc()
                    for vc in range(NJ):
                        nc.tensor.matmul(
                            pd,
                            oT_t[:, vc * S + st * 128: vc * S + (st + 1) * 128],
                            wo_t[:, vc * D: vc * D + 512],
                            start=(vc == 0), stop=(vc == NJ - 1))
                    (cp or nc.vector.tensor_copy)(ot[0][:, 0:512], pd)
                    nc.sync.dma_start(out[st * 128:(st + 1) * 128, 0:512],
                                      ot[0][:, 0:512])

                def emit2(st=st, ot=ot):
                    pd = alloc()
                    for vc in range(NJ):
                        nc.tensor.matmul(
                            pd,
                            oT_t[:, vc * S + st * 128: vc * S + (st + 1) * 128],
                            wo_t[:, vc * D + 512: vc * D + 1024],
                            start=(vc == 0), stop=(vc == NJ - 1))
                    (cp or nc.vector.tensor_copy)(ot[0][:, 512:1024], pd)
                    nc.sync.dma_start(out[st * 128:(st + 1) * 128, 512:1024],
                                      ot[0][:, 512:1024])

                us.append(emit)
                us.append(emit2)
            return us

        # ---- phase C
        pending_tr = [None]   # deferred transpose emitter (1-pair pipeline)

        def flush_tr():
            if pending_tr[0] is not None:
                pending_tr[0]()
                pending_tr[0] = None

        def phase_c(qt, units):
            n_pair = NH * (2 * qt + 2)
            frac = len(units) / n_pair if units else 0.0
            ui = iter(units)
            acc = 0.99
            a_pair = [None, None]
            po_h = {}
            av_q = []

            def make_tr(at, hp):
                # both heads' normalized outputs share one [128, 4*128]
                # tile (even head cols 0:64 of each block, odd 64:128), so
                # ONE [128,128] transpose per q-subtile covers the pair —
                # transpose cost depends only on output free size.
                def emit_tr():
                    pt = pV.tile([128, 512], BF16, tag="po",
                                 name="pt", padded_shape=[128, 1024])
                    for qsub in range(4):
                        nc.tensor.transpose(
                            pt[:, qsub * 128:(qsub + 1) * 128],
                            at[:, qsub * 128:(qsub + 1) * 128],
                            ident_t[:])
                    if qt == 3 and hp == NJ - 1:
                        for q4 in range(4):   # quartered: D(3) starts sooner
                            nc.vector.tensor_copy(
                                oT_t[:, hp * S + qt * 512 + q4 * 128:
                                     hp * S + qt * 512 + (q4 + 1) * 128],
                                pt[:, q4 * 128:(q4 + 1) * 128])
                    else:
                        nc.vector.tensor_copy(
                            oT_t[:, hp * S + qt * 512:
                                 hp * S + (qt + 1) * 512],
                            pt[:, 0:512])
                return emit_tr

            def emit_norm(h):
                hp, h2 = h // 2, h % 2
                po = po_h.pop(h)
                if h2 == 0:
                    a_pair[0] = ap_.tile([128, 4 * 128], BF16, tag="a",
                                         name="a")
                at = a_pair[0]
                if qt == 3 and h == NH - 1:
                    # tail: normalize straight from PSUM on DVE
                    po_r = po[:].rearrange("p (q w) -> p q w", q=4)
                    r = pf.tile([128, 4], F32, tag="r", name="r")
                    nc.vector.reciprocal(r[:], po_r[:, :, DK])
                    for qsub in range(4):
                        o = qsub * 128 + h2 * DK
                        nc.vector.tensor_scalar(
                            at[:, o:o + DK],
                            po_r[:, qsub, 0:DK], r[:, qsub:qsub + 1], None,
                            op0=MULT)
                else:
                    pof = pf.tile([128, 4 * VW], F32, tag="pof", name="pof")
                    nc.vector.tensor_copy(pof[:], po[:])
                    pof_r = pof[:].rearrange("p (q w) -> p q w", q=4)
                    for qsub in range(4):
                        o = qsub * 128 + h2 * DK
                        nc.gpsimd.normalize_recip(
                            at[:, o:o + DK],
                            pof_r[:, qsub, 0:DK], pof_r[:, qsub, DK:VW])
                if h2 == 1:
                    tr = make_tr(at, hp)
                    flush_tr()          # previous pair, one pair deferred
                    pending_tr[0] = tr

            for h in range(NH):
                hp, h2 = h // 2, h % 2
                lo = 64 * h2
                qcol = hp * S + qt * 512
                po = pV.tile([128, 4 * VW], F32, tag="po", name="po",
                             padded_shape=[128, 512])
                po_h[h] = po
                for kp in range(2 * qt + 2):
                    ps = pS.tile([128, 1024], F32, tag="sc", name="ps")
                    e = ep.tile([128, 1024], BF16, tag="e", name="e")
                    # column base of k2's q-range in ps/e: diagonal pairs
                    # are COMPACTED so one contiguous exp covers both
                    # halves with no junk columns.
                    dd0 = 2 * kp - 4 * qt
                    if dd0 == 0:          # diag pair 1: [0:512]+[512:896]
                        base = (0, 512 - 128)
                        erng = (0, 896)
                    elif dd0 == 2:        # diag pair 2: [256:512]+[512:640]
                        base = (0, 512 - 384)
                        erng = (256, 640)
                    else:                 # full pair
                        base = (0, 512)
                        erng = (0, 1024)
                    for k2 in range(2):
                        kt = 2 * kp + k2
                        off = max(kt - 4 * qt, 0) * 128
                        nc.tensor.matmul(
                            ps[:, base[k2] + off: base[k2] + 512],
                            kT_t[lo:lo + 64,
                                 hp * S + kt * 128: hp * S + (kt + 1) * 128],
                            qT_t[lo:lo + 64, qcol + off: qcol + 512],
                            start=True, stop=True)
                    nc.scalar.activation(e[:, erng[0]:erng[1]],
                                         ps[:, erng[0]:erng[1]],
                                         EXP, scale=0.125)
                    for k2 in range(2):
                        dd = 2 * kp + k2 - 4 * qt
                        if dd >= 0:
                            o = base[k2] + dd * 128
                            nc.gpsimd.tensor_tensor(
                                e[:, o:o + 128], e[:, o:o + 128],
                                mask_t[:], op=MULT)

                    def make_av(e=e, kp=kp, h=h, po=po, base=base):
                        def emit_av():
                            for k2 in range(2):
                                kt = 2 * kp + k2
                                dd = kt - 4 * qt
                                for qsub in range(max(dd, 0), 4):
                                    nc.tensor.matmul(
                                        po[:, qsub * VW: qsub * VW + VW],
                                        e[:, base[k2] + qsub * 128:
                                           base[k2] + (qsub + 1) * 128],
                                        v_t[:, (kt * NH + h) * VW:
                                            (kt * NH + h + 1) * VW],
                                        start=(kt == 0 and qsub == 0),
                                        stop=(kt == 4 * qt + 3 and qsub == 3),
                                        skip_group_check=True)
                        return emit_av
                    acc += frac
                    while acc >= 1.0:
                        acc -= 1.0
                        u = next(ui, None)
                        if u is not None:
                            u()
                    if len(av_q) >= (3 if qt >= 1 else 2):
                        av_q.pop(0)()
                    if kp == (2 if qt >= 1 else 1) and h > 0:
                        emit_norm(h - 1)
                    av_q.append(make_av())
            for av in av_q:
                av()
            av_q.clear()
            emit_norm(NH - 1)
            for u in ui:
                u()

        # ---- main schedule
        def ps_full():
            return pS.tile([128, 1024], F32, tag="sc", name="pp")[:, 0:512]

        b0_qk(wq_t, bq_t, qT_t)
        b0_qk(wk_t, bk_t, kT_t)
        b0_v()
        for qt in range(NSC):
            if qt < 3:
                units = b_units(qt + 1, ps_full)
            else:
                units = (d_units(0, ps_full) + d_units(1, ps_full)
                         + d_units(2, ps_full))
            phase_c(qt, units)
        flush_tr()
        for u in d_units(3, ps_full, cp=nc.scalar.copy):
            u()

    nc.compile()
    return nc


# ----------------------------------------------------------------- host side

_NC_CACHE = None


def _get_nc():
    global _NC_CACHE
    if _NC_CACHE is None:
        _NC_CACHE = build_mha()
    return _NC_CACHE


def shard_inputs(x, Wq, bq, Wk, bk, Wv, bv, Wo, bo):
    import ml_dtypes
    bf16 = ml_dtypes.bfloat16
    mask = np.triu(np.ones((128, 128), np.float32)).astype(bf16)
    ident = np.eye(128, dtype=np.float32).astype(bf16)
    x = np.asarray(x, dtype=np.float32)
    xTs = [np.ascontiguousarray(x[b].T).astype(bf16) for b in range(4)]
    Wq, Wk, Wv, Wo = (np.asarray(w, np.float32) for w in (Wq, Wk, Wv, Wo))
    bq, bk, bv = (np.asarray(b_, np.float32) for b_ in (bq, bk, bv))
    maps = []
    for c in range(8):
        b, g = c // 2, c % 2
        sl = slice(g * JC, (g + 1) * JC)
        maps.append({
            "xT": xTs[b],
            "wq": np.ascontiguousarray(Wq[:, sl]).astype(bf16),
            "wk": np.ascontiguousarray(Wk[:, sl]).astype(bf16),
            "wv": np.ascontiguousarray(Wv[:, sl]).astype(bf16),
            "wo": np.ascontiguousarray(Wo[sl, :]).astype(bf16),
            "bq": np.ascontiguousarray(bq[sl].reshape(NJ, 128).T),
            "bk": np.ascontiguousarray(bk[sl].reshape(NJ, 128).T),
            "bvb": np.broadcast_to(bv[sl], (128, JC)).copy(),
            "mask": mask,
            "ident": ident,
        })
    return maps


def kernel(x, Wq, bq, Wk, bk, Wv, bv, Wo, bo):
    """Full-input entry point: shard across 8 NeuronCores, run, gather."""
    from concourse.bass_utils import run_bass_kernel_spmd

    nc = _get_nc()
    in_maps = shard_inputs(x, Wq, bq, Wk, bk, Wv, bv, Wo, bo)
    res = run_bass_kernel_spmd(nc, in_maps, list(range(8)))
    bo = np.asarray(bo, dtype=np.float32)
    out = np.empty((4, S, D), dtype=np.float32)
    for b in range(4):
        out[b] = (res.results[2 * b]["out"].astype(np.float32)
                  + res.results[2 * b + 1]["out"].astype(np.float32) + bo)
    return out
